# revision 11
# baseline (speedup 1.0000x reference)
"""Linformer self-attention on 8 Trainium2 NeuronCores.

Sharding: core = (batch b, head-group g) with b = core//2, g = core%2.
Each core computes attention for batch b and its 8 heads (512 of the 1024
channels), then a row-sharded W_out matmul producing a partial output in
transposed [1024, 4096] fp16 layout; the host sums the two partials per
batch in f32, transposes once, and adds b_out.

Key algebraic restructure vs the naive dataflow (Linformer associativity):
  k_proj = E^T (x Wk) = (E^T x) Wk   and likewise for v_proj.
Computing xE^T = x^T E first (shared by k and v) cuts the projection phase
from ~328k to ~82k PE cycles.

Per-core dataflow:
  FRONT: stream xn (natural [n,d] tiles) + E tiles; accumulate
    xET[d,kr] = sum_n x[n,d] E[n,kr] in PSUM; PE-transpose each xn tile to
    xnT for the q path; every 4 tiles emit one slab's qT = Wq^T x^T
    (Wq pre-scaled by 1/sqrt(hd) on host); then kpT = Wk^T xET and
    vp = xET^T-slices @ Wv.
  HEADS (per slab s, head h, software-pipelined): scores in natural
    layout [n,kr] via one PE pass; row max via free-axis reduce (negated);
    exp on ScalarE with per-partition bias; denominator via free-axis
    reduce_sum; tiny [128,4] reciprocal; normalize U (per-partition
    scalar mult); PE-transpose of normalized U (fp16); AV matmul;
    W_out chunks of the previous slab interleaved between heads.

Precision: q/k score chain fully f32r; U/vp/attn-out/W_out/output fp16.
"""

import os
import numpy as np

import concourse.bacc as bacc
import concourse.tile as tile
from concourse import mybir
from concourse.bass_utils import run_bass_kernel_spmd

F32 = mybir.dt.float32
F16 = mybir.dt.float16
F32R = mybir.dt.float32r
EXP = mybir.ActivationFunctionType.Exp
AXX = mybir.AxisListType.X

DIM, SEQ, KR, HD = 1024, 4096, 256, 64
CG = 512               # channels per head-group (8 heads x 64)
NSLAB = 512
SLABS = SEQ // NSLAB   # 8
NT = SEQ // 128        # 32 natural n-tiles
DC = DIM // 128        # 8 d-tiles
SCALE = HD ** -0.5

_cache = {}


def build_program():
    nc = bacc.Bacc("TRN2", target_bir_lowering=False, debug=False, num_devices=8)

    xn_d = nc.dram_tensor("xn", [NT, 128, DIM], F32R, kind="ExternalInput")
    Ed = nc.dram_tensor("E", [NT, 128, KR], F32R, kind="ExternalInput")
    Wq = nc.dram_tensor("Wq", [DC, 128, CG], F32R, kind="ExternalInput")
    Wk = nc.dram_tensor("Wk", [DC, 128, CG], F32R, kind="ExternalInput")
    Wv = nc.dram_tensor("Wv", [DC, 128, CG], F32R, kind="ExternalInput")
    Wo = nc.dram_tensor("Wo", [CG // 128, 128, DIM], F16, kind="ExternalInput")
    id32_d = nc.dram_tensor("id32", [128, 128], F32R, kind="ExternalInput")
    id16_d = nc.dram_tensor("id16", [128, 128], F16, kind="ExternalInput")
    out_d = nc.dram_tensor("out", [DIM, SEQ], F16, kind="ExternalOutput")
    dbg = os.environ.get("KERNEL_DEBUG", "0") == "1"
    if dbg:
        dbg_xET = nc.dram_tensor("dbg_xET", [128, DC, KR], F32R, kind="ExternalOutput")
        dbg_kpT = nc.dram_tensor("dbg_kpT", [128, 4, KR], F32R, kind="ExternalOutput")
        dbg_vp = nc.dram_tensor("dbg_vp", [128, 2, CG], F16, kind="ExternalOutput")
        dbg_qt = nc.dram_tensor("dbg_qt", [128, 4, NSLAB], F32R, kind="ExternalOutput")

    mm = nc.tensor.matmul

    with tile.TileContext(nc) as tc:
        with tc.tile_pool(name="const", bufs=1) as const:
            wq_sb = const.tile([128, DC, CG], F32R)
            wo_sb = const.tile([128, 4, DIM], F16)
            id32_sb = const.tile([128, 128], F32R)
            id16_sb = const.tile([128, 128], F16)
            xET_sb = const.tile([128, DC, KR], F32R)   # x^T E  [d, kr]
            kpT_sb = const.tile([128, 4, KR], F32R)    # (kp)^T [c, kr]
            vp_sb = const.tile([128, 2, CG], F16)      # vp     [kr, c]
            qt = const.tile([128, 4, SLABS, NSLAB], F32R)  # q^T, all slabs

            for dc in range(DC):
                nc.sync.dma_start(out=wq_sb[:, dc, :], in_=Wq[dc])
            nc.sync.dma_start(out=id32_sb, in_=id32_d[:, :])
            nc.sync.dma_start(out=id16_sb, in_=id16_d[:, :])
            for ct in range(4):
                nc.sync.dma_start(out=wo_sb[:, ct, :], in_=Wo[ct])

            # ---------------- FRONT: xET accumulate + xnT transposes + qT ----
            with tc.tile_pool(name="frA", bufs=1) as frA, \
                 tc.tile_pool(name="psQ", bufs=1, space="PSUM") as psQ:
                wk_sb = frA.tile([128, DC, CG], F32R)
                for dc in range(DC):
                    nc.sync.dma_start(out=wk_sb[:, dc, :], in_=Wk[dc])
                wv_sb = frA.tile([128, DC, CG], F32R)
                for dc in range(DC):
                    nc.sync.dma_start(out=wv_sb[:, dc, :], in_=Wv[dc])

                def rr_copy(i, out, in_):
                    eng = (nc.vector.tensor_copy, nc.scalar.copy)[i % 2]
                    eng(out, in_)

                with tc.tile_pool(name="psE", bufs=1, space="PSUM") as psE:
                    xET_ps = psE.tile([128, DC, KR], F32)  # 4 banks
                    for nt in range(NT):
                        xt = frA.tile([128, DIM], F32R, tag="xn", bufs=3,
                                      name=f"xt_{nt}")
                        nc.sync.dma_start(out=xt, in_=xn_d[nt])
                        et = frA.tile([128, KR], F32R, tag="et", bufs=3,
                                      name=f"et_{nt}")
                        nc.sync.dma_start(out=et, in_=Ed[nt])
                        for dsub in range(DC):
                            mm(xET_ps[:, dsub, :],
                               lhsT=xt[:, dsub * 128:(dsub + 1) * 128], rhs=et,
                               start=(nt == 0 and dsub % 2 == 0),
                               stop=(nt == NT - 1))
                        # transpose this n-tile for the q path
                        s, j = nt // 4, nt % 4
                        if j == 0:
                            xnT = frA.tile([128, DC, 4, 128], F32R, tag="xnT",
                                           bufs=2, name=f"xnT_{s}")
                        for half in range(2):
                            tp = psQ.tile([128, 4, 128], F32R, tag="tp", bufs=2,
                                          name=f"tp_{nt}_{half}")
                            for i in range(4):
                                dsub = half * 4 + i
                                mm(tp[:, i, :],
                                   lhsT=xt[:, dsub * 128:(dsub + 1) * 128],
                                   rhs=id32_sb, is_transpose=True,
                                   start=(i == 0), stop=(i == 3))
                            rr_copy(2 * nt + half,
                                    xnT[:, half * 4:(half + 1) * 4, j, :], tp)
                        if j == 3:
                            for ct in range(4):
                                q_ps = psQ.tile([128, NSLAB], F32, tag="qps",
                                                bufs=2, name=f"qps_{s}_{ct}")
                                for dc in range(DC):
                                    mm(q_ps,
                                       lhsT=wq_sb[:, dc, ct * 128:(ct + 1) * 128],
                                       rhs=xnT[:, dc, :, :],
                                       start=(dc == 0), stop=(dc == DC - 1))
                                rr_copy(ct, qt[:, ct, s, :], q_ps)
                    for d2 in range(0, DC, 2):
                        rr_copy(d2 // 2, xET_sb[:, d2:d2 + 2, :],
                                xET_ps[:, d2:d2 + 2, :])

                # kpT / vp from xET (reuses the 4 banks freed by psE)
                with tc.tile_pool(name="psKV", bufs=1, space="PSUM") as psKV:
                    kpT_ps = psKV.tile([128, 4, KR], F32)
                    for dc in range(DC):
                        for ct in range(4):
                            mm(kpT_ps[:, ct, :],
                               lhsT=wk_sb[:, dc, ct * 128:(ct + 1) * 128],
                               rhs=xET_sb[:, dc, :],
                               start=(dc == 0 and ct % 2 == 0),
                               stop=(dc == DC - 1))
                    nc.vector.tensor_copy(kpT_sb, kpT_ps)
                    vp_ps = psKV.tile([128, 2, CG], F32)
                    for dc in range(DC):
                        for krt in range(2):
                            mm(vp_ps[:, krt, :],
                               lhsT=xET_sb[:, dc, krt * 128:(krt + 1) * 128],
                               rhs=wv_sb[:, dc, :],
                               start=(dc == 0), stop=(dc == DC - 1))
                    nc.vector.tensor_copy(vp_sb, vp_ps)

            if dbg:
                nc.sync.dma_start(out=dbg_xET[:, :, :], in_=xET_sb)
                nc.sync.dma_start(out=dbg_kpT[:, :, :], in_=kpT_sb)
                nc.sync.dma_start(out=dbg_vp[:, :, :], in_=vp_sb)
                nc.sync.dma_start(out=dbg_qt[:, :, :], in_=qt[:, :, 0, :])

            # ---------------- HEADS epoch (software-pipelined) --------------
            with tc.tile_pool(name="hp", bufs=1) as hp, \
                 tc.tile_pool(name="psH", bufs=1, space="PSUM") as psH:
                outU = hp.tile([128, 4, 2, NSLAB], F16)  # 2-slab ring

                NSTEP = SLABS * 8
                state = {}

                def stage_nat(t):
                    s, h = t // 8, t % 8
                    hp_, ct_h = (h % 2) * 64, h // 2
                    kph = kpT_sb[hp_:hp_ + 64, ct_h, :]
                    nat = [None, None]
                    mrows = hp.tile([128, 4], F32, tag="mrows", bufs=3,
                                    name=f"mrows_{t}")
                    U_nat = hp.tile([128, 4, KR], F16, tag="unat", bufs=2,
                                    name=f"unat_{t}")
                    for hf in range(2):
                        natp = psH.tile([128, 2, KR], F32, tag="nat", bufs=2,
                                        name=f"nat_{t}_{hf}")
                        for i in range(2):
                            ns = hf * 2 + i
                            qh = qt[hp_:hp_ + 64, ct_h, s,
                                    ns * 128:(ns + 1) * 128]
                            mm(natp[:, i, :], lhsT=qh, rhs=kph,
                               start=(i == 0), stop=(i == 1))
                        nc.vector.reduce_max(mrows[:, 2 * hf:2 * hf + 2], natp,
                                             axis=AXX, negate=True)
                        for i in range(2):
                            ns = hf * 2 + i
                            nc.scalar.activation(U_nat[:, ns, :], natp[:, i, :],
                                                 EXP, bias=mrows[:, ns:ns + 1],
                                                 scale=1.0)
                        nat[hf] = natp
                    denom = hp.tile([128, 4], F32, tag="denom", bufs=2,
                                    name=f"denom_{t}")
                    srecip = hp.tile([128, 4], F32, tag="srecip", bufs=2,
                                     name=f"srecip_{t}")
                    nc.vector.reduce_sum(denom, U_nat, axis=AXX)
                    nc.vector.reciprocal(srecip, denom)
                    U_norm = hp.tile([128, 4, KR], F16, tag="unorm", bufs=3,
                                     name=f"unorm_{t}")
                    for ns in range(4):
                        nc.gpsimd.tensor_scalar_mul(U_norm[:, ns, :],
                                                    U_nat[:, ns, :],
                                                    srecip[:, ns:ns + 1])
                    return U_norm

                def stage_T(t):
                    U_norm = state[t]["U_norm"]
                    UT_ps = psH.tile([128, 2, NSLAB], F16, tag="utps", bufs=2,
                                     name=f"utps_{t}")
                    for ns in range(4):
                        for kb in range(2):
                            mm(UT_ps[:, kb, ns * 128:(ns + 1) * 128],
                               lhsT=U_norm[:, ns, kb * 128:(kb + 1) * 128],
                               rhs=id16_sb, is_transpose=True,
                               start=(ns == 0 and kb == 0),
                               stop=(ns == 3 and kb == 1))
                    UT_sb = hp.tile([128, 2, NSLAB], F16, tag="utsb", bufs=2,
                                    name=f"utsb_{t}")
                    nc.vector.tensor_copy(UT_sb, UT_ps)
                    return UT_sb

                def stage_av(t):
                    s, h = t // 8, t % 8
                    hp_, ct_h = (h % 2) * 64, h // 2
                    UT_sb = state[t]["UT_sb"]
                    av_ps = psH.tile([128, NSLAB], F32, tag="av", bufs=2,
                                     name=f"av_{t}")
                    for krt in range(2):
                        mm(av_ps[hp_:hp_ + 64, :],
                           lhsT=vp_sb[:, krt, h * 64:(h + 1) * 64],
                           rhs=UT_sb[:, krt, :],
                           start=(krt == 0), stop=(krt == 1))
                    nc.scalar.copy(outU[hp_:hp_ + 64, ct_h, s % 2, :],
                                   av_ps[hp_:hp_ + 64, :])

                def stage_C(s, jc):
                    f_ps = psH.tile([128, NSLAB], F32, tag="fps", bufs=2,
                                    name=f"fps_{s}_{jc}")
                    for ct in range(4):
                        mm(f_ps, lhsT=wo_sb[:, ct, jc * 128:(jc + 1) * 128],
                           rhs=outU[:, ct, s % 2, :],
                           start=(ct == 0), stop=(ct == 3))
                    ot = hp.tile([128, NSLAB], F16, tag="ot", bufs=4,
                                 name=f"ot_{s}_{jc}")
                    nc.scalar.copy(ot, f_ps)
                    nc.sync.dma_start(
                        out=out_d[jc * 128:(jc + 1) * 128,
                                  s * NSLAB:(s + 1) * NSLAB], in_=ot)

                # C(s-1) chunk schedule: 8 jc chunks over steps h=3..7 of slab s
                c_sched = {3: [0, 1], 4: [2, 3], 5: [4], 6: [5], 7: [6, 7]}

                for t in range(NSTEP):
                    s, h = t // 8, t % 8
                    state[t] = {}
                    state[t]["U_norm"] = stage_nat(t)
                    if t - 2 >= 0:
                        state[t - 2]["UT_sb"] = stage_T(t - 2)
                    if t - 3 >= 0:
                        stage_av(t - 3)
                    if s >= 1:
                        for jc in c_sched.get(h, []):
                            stage_C(s - 1, jc)
                # epilogue
                for t in (NSTEP - 2, NSTEP - 1):
                    state[t]["UT_sb"] = stage_T(t)
                for t in (NSTEP - 3, NSTEP - 2, NSTEP - 1):
                    stage_av(t)
                for jc in range(8):
                    stage_C(SLABS - 1, jc)

    nc.compile()
    return nc


def kernel(x, W_qkv, E, W_out, b_out):
    x = np.ascontiguousarray(np.asarray(x, dtype=np.float32))
    W_qkv = np.asarray(W_qkv, dtype=np.float32)
    E_np = np.asarray(E, dtype=np.float32)
    W_out = np.asarray(W_out, dtype=np.float32)
    b_out = np.asarray(b_out, dtype=np.float32)

    if "nc" not in _cache:
        _cache["nc"] = build_program()
    nc = _cache["nc"]

    E_t = np.ascontiguousarray(E_np.reshape(NT, 128, KR))
    id32 = np.eye(128, dtype=np.float32)
    id16 = np.eye(128, dtype=np.float16)
    in_maps = []
    for core in range(8):
        b, g = core // 2, core % 2
        cols = slice(g * CG, (g + 1) * CG)
        xn_t = x[b].reshape(NT, 128, DIM)
        Wq_t = np.ascontiguousarray(
            (W_qkv[:, 0 * DIM:1 * DIM][:, cols] * SCALE)).reshape(DC, 128, CG)
        Wk_t = np.ascontiguousarray(W_qkv[:, 1 * DIM:2 * DIM][:, cols]).reshape(
            DC, 128, CG)
        Wv_t = np.ascontiguousarray(W_qkv[:, 2 * DIM:3 * DIM][:, cols]).reshape(
            DC, 128, CG)
        Wo_t = np.ascontiguousarray(
            W_out[g * CG:(g + 1) * CG, :].astype(np.float16)).reshape(
            CG // 128, 128, DIM)
        in_maps.append({
            "xn": xn_t, "E": E_t, "Wq": Wq_t, "Wk": Wk_t, "Wv": Wv_t,
            "Wo": Wo_t, "id32": id32, "id16": id16,
        })

    trace = bool(int(os.environ.get("KERNEL_TRACE", "0")))
    res = run_bass_kernel_spmd(nc, in_maps, core_ids=list(range(8)), trace=trace)
    _cache["last_results"] = res

    # partials come back transposed [DIM, SEQ] fp16; sum per batch in f32
    accT = np.zeros((4, DIM, SEQ), dtype=np.float32)
    for core in range(8):
        accT[core // 2] += res.results[core]["out"].astype(np.float32)
    out = np.ascontiguousarray(accT.transpose(0, 2, 1))
    out += b_out[None, None, :]
    return out


# revision 12
# speedup vs baseline: 2.4579x; 2.4579x over previous
"""Linformer self-attention on 8 Trainium2 NeuronCores.

Sharding: core = (batch b, head-group g) with b = core//2, g = core%2.
Each core computes attention for batch b and its 8 heads (512 of the 1024
channels), then a row-sharded W_out matmul producing a partial output in
transposed [1024, 4096] fp16 layout; the host sums the two partials per
batch in f32, transposes once, and adds b_out.

Key algebraic restructure vs the naive dataflow (Linformer associativity):
  k_proj = E^T (x Wk) = (E^T x) Wk   and likewise for v_proj.
Computing xE^T = x^T E first (shared by k and v) cuts the projection phase
from ~328k to ~82k PE cycles.

Per-core dataflow:
  FRONT: stream xn (natural [n,d] tiles) + E tiles; accumulate
    xET[d,kr] = sum_n x[n,d] E[n,kr] in PSUM; PE-transpose each xn tile to
    xnT for the q path; every 4 tiles emit one slab's qT = Wq^T x^T
    (Wq pre-scaled by 1/sqrt(hd) on host); then kpT = Wk^T xET and
    vp = xET^T-slices @ Wv.
  HEADS (per slab s, head h, software-pipelined): scores in natural
    layout [n,kr] via one PE pass; row max via free-axis reduce (negated);
    exp on ScalarE with per-partition bias; denominator via free-axis
    reduce_sum; tiny [128,4] reciprocal; normalize U (per-partition
    scalar mult); PE-transpose of normalized U (fp16); AV matmul;
    W_out chunks of the previous slab interleaved between heads.

Precision: q/k score chain fully f32r; U/vp/attn-out/W_out/output fp16.
"""

import os
import numpy as np

import concourse.bacc as bacc
import concourse.tile as tile
from concourse import mybir
from concourse.bass_utils import run_bass_kernel_spmd

F32 = mybir.dt.float32
F16 = mybir.dt.float16
F32R = mybir.dt.float32r
EXP = mybir.ActivationFunctionType.Exp
AXX = mybir.AxisListType.X

DIM, SEQ, KR, HD = 1024, 4096, 256, 64
CG = 512               # channels per head-group (8 heads x 64)
NSLAB = 512
SLABS = SEQ // NSLAB   # 8
NT = SEQ // 128        # 32 natural n-tiles
DC = DIM // 128        # 8 d-tiles
SCALE = HD ** -0.5

_cache = {}


def build_program():
    nc = bacc.Bacc("TRN2", target_bir_lowering=False, debug=False, num_devices=8)

    xn_d = nc.dram_tensor("xn", [NT, 128, DIM], F32R, kind="ExternalInput")
    Ed = nc.dram_tensor("E", [NT, 128, KR], F32R, kind="ExternalInput")
    Wq = nc.dram_tensor("Wq", [DC, 128, CG], F32R, kind="ExternalInput")
    Wk = nc.dram_tensor("Wk", [DC, 128, CG], F32R, kind="ExternalInput")
    Wv = nc.dram_tensor("Wv", [DC, 128, CG], F32R, kind="ExternalInput")
    Wo = nc.dram_tensor("Wo", [CG // 128, 128, DIM], F16, kind="ExternalInput")
    id32_d = nc.dram_tensor("id32", [128, 128], F32R, kind="ExternalInput")
    id16_d = nc.dram_tensor("id16", [128, 128], F16, kind="ExternalInput")
    out_d = nc.dram_tensor("out", [DIM, SEQ], F16, kind="ExternalOutput")
    dbg = os.environ.get("KERNEL_DEBUG", "0") == "1"
    if dbg:
        dbg_xET = nc.dram_tensor("dbg_xET", [128, DC, KR], F32R, kind="ExternalOutput")
        dbg_kpT = nc.dram_tensor("dbg_kpT", [128, 4, KR], F32R, kind="ExternalOutput")
        dbg_vp = nc.dram_tensor("dbg_vp", [128, 2, CG], F16, kind="ExternalOutput")
        dbg_qt = nc.dram_tensor("dbg_qt", [128, 4, NSLAB], F32R, kind="ExternalOutput")

    mm = nc.tensor.matmul

    with tile.TileContext(nc) as tc:
        with tc.tile_pool(name="const", bufs=1) as const:
            wq_sb = const.tile([128, DC, CG], F32R)
            wo_sb = const.tile([128, 4, DIM], F16)
            id32_sb = const.tile([128, 128], F32R)
            id16_sb = const.tile([128, 128], F16)
            xET_sb = const.tile([128, DC, KR], F32R)   # x^T E  [d, kr]
            kpT_sb = const.tile([128, 4, KR], F32R)    # (kp)^T [c, kr]
            vp_sb = const.tile([128, 2, CG], F16)      # vp     [kr, c]
            qt = const.tile([128, 4, SLABS, NSLAB], F32R)  # q^T, all slabs

            for dc in range(DC):
                nc.sync.dma_start(out=wq_sb[:, dc, :], in_=Wq[dc])
            nc.sync.dma_start(out=id32_sb, in_=id32_d[:, :])
            nc.sync.dma_start(out=id16_sb, in_=id16_d[:, :])
            for ct in range(4):
                nc.sync.dma_start(out=wo_sb[:, ct, :], in_=Wo[ct])

            # ---------------- FRONT: xET accumulate + xnT transposes + qT ----
            with tc.tile_pool(name="frA", bufs=1) as frA, \
                 tc.tile_pool(name="psQ", bufs=1, space="PSUM") as psQ:
                wk_sb = frA.tile([128, DC, CG], F32R)
                for dc in range(DC):
                    nc.sync.dma_start(out=wk_sb[:, dc, :], in_=Wk[dc])
                wv_sb = frA.tile([128, DC, CG], F32R)
                for dc in range(DC):
                    nc.sync.dma_start(out=wv_sb[:, dc, :], in_=Wv[dc])

                def rr_copy(i, out, in_):
                    eng = (nc.vector.tensor_copy, nc.scalar.copy)[i % 2]
                    eng(out, in_)

                with tc.tile_pool(name="psE", bufs=1, space="PSUM") as psE:
                    xET_ps = psE.tile([128, DC, KR], F32)  # 4 banks
                    for nt in range(NT):
                        xt = frA.tile([128, DIM], F32R, tag="xn", bufs=3,
                                      name=f"xt_{nt}")
                        nc.sync.dma_start(out=xt, in_=xn_d[nt])
                        et = frA.tile([128, KR], F32R, tag="et", bufs=3,
                                      name=f"et_{nt}")
                        nc.sync.dma_start(out=et, in_=Ed[nt])
                        for dsub in range(DC):
                            mm(xET_ps[:, dsub, :],
                               lhsT=xt[:, dsub * 128:(dsub + 1) * 128], rhs=et,
                               start=(nt == 0 and dsub % 2 == 0),
                               stop=(nt == NT - 1))
                        # transpose this n-tile for the q path
                        s, j = nt // 4, nt % 4
                        if j == 0:
                            xnT = frA.tile([128, DC, 4, 128], F32R, tag="xnT",
                                           bufs=2, name=f"xnT_{s}")
                        for half in range(2):
                            tp = psQ.tile([128, 4, 128], F32R, tag="tp", bufs=2,
                                          name=f"tp_{nt}_{half}")
                            for i in range(4):
                                dsub = half * 4 + i
                                mm(tp[:, i, :],
                                   lhsT=xt[:, dsub * 128:(dsub + 1) * 128],
                                   rhs=id32_sb, is_transpose=True,
                                   start=(i == 0), stop=(i == 3))
                            rr_copy(2 * nt + half,
                                    xnT[:, half * 4:(half + 1) * 4, j, :], tp)
                        if j == 3:
                            for ct in range(4):
                                q_ps = psQ.tile([128, NSLAB], F32, tag="qps",
                                                bufs=2, name=f"qps_{s}_{ct}")
                                for dc in range(DC):
                                    mm(q_ps,
                                       lhsT=wq_sb[:, dc, ct * 128:(ct + 1) * 128],
                                       rhs=xnT[:, dc, :, :],
                                       start=(dc == 0), stop=(dc == DC - 1))
                                rr_copy(ct, qt[:, ct, s, :], q_ps)
                    for d2 in range(0, DC, 2):
                        rr_copy(d2 // 2, xET_sb[:, d2:d2 + 2, :],
                                xET_ps[:, d2:d2 + 2, :])

                # kpT / vp from xET (reuses the 4 banks freed by psE)
                with tc.tile_pool(name="psKV", bufs=1, space="PSUM") as psKV:
                    kpT_ps = psKV.tile([128, 4, KR], F32)
                    for dc in range(DC):
                        for ct in range(4):
                            mm(kpT_ps[:, ct, :],
                               lhsT=wk_sb[:, dc, ct * 128:(ct + 1) * 128],
                               rhs=xET_sb[:, dc, :],
                               start=(dc == 0 and ct % 2 == 0),
                               stop=(dc == DC - 1))
                    nc.vector.tensor_copy(kpT_sb, kpT_ps)
                    vp_ps = psKV.tile([128, 2, CG], F32)
                    for dc in range(DC):
                        for krt in range(2):
                            mm(vp_ps[:, krt, :],
                               lhsT=xET_sb[:, dc, krt * 128:(krt + 1) * 128],
                               rhs=wv_sb[:, dc, :],
                               start=(dc == 0), stop=(dc == DC - 1))
                    nc.vector.tensor_copy(vp_sb, vp_ps)

            if dbg:
                nc.sync.dma_start(out=dbg_xET[:, :, :], in_=xET_sb)
                nc.sync.dma_start(out=dbg_kpT[:, :, :], in_=kpT_sb)
                nc.sync.dma_start(out=dbg_vp[:, :, :], in_=vp_sb)
                nc.sync.dma_start(out=dbg_qt[:, :, :], in_=qt[:, :, 0, :])

            # ---------------- HEADS epoch (software-pipelined) --------------
            with tc.tile_pool(name="hp", bufs=1) as hp, \
                 tc.tile_pool(name="psH", bufs=1, space="PSUM") as psH:
                outU = hp.tile([128, 4, 2, NSLAB], F16)  # 2-slab ring

                NSTEP = SLABS * 8
                state = {}

                def stage_nat(t):
                    s, h = t // 8, t % 8
                    hp_, ct_h = (h % 2) * 64, h // 2
                    kph = kpT_sb[hp_:hp_ + 64, ct_h, :]
                    nat = [None, None]
                    mrows = hp.tile([128, 4], F32, tag="mrows", bufs=3,
                                    name=f"mrows_{t}")
                    U_nat = hp.tile([128, 4, KR], F16, tag="unat", bufs=2,
                                    name=f"unat_{t}")
                    for hf in range(2):
                        natp = psH.tile([128, 2, KR], F32, tag="nat", bufs=2,
                                        name=f"nat_{t}_{hf}")
                        for i in range(2):
                            ns = hf * 2 + i
                            qh = qt[hp_:hp_ + 64, ct_h, s,
                                    ns * 128:(ns + 1) * 128]
                            mm(natp[:, i, :], lhsT=qh, rhs=kph,
                               start=(i == 0), stop=(i == 1))
                        nc.vector.reduce_max(mrows[:, 2 * hf:2 * hf + 2], natp,
                                             axis=AXX, negate=True)
                        for i in range(2):
                            ns = hf * 2 + i
                            nc.scalar.activation(U_nat[:, ns, :], natp[:, i, :],
                                                 EXP, bias=mrows[:, ns:ns + 1],
                                                 scale=1.0)
                        nat[hf] = natp
                    denom = hp.tile([128, 4], F32, tag="denom", bufs=2,
                                    name=f"denom_{t}")
                    srecip = hp.tile([128, 4], F32, tag="srecip", bufs=2,
                                     name=f"srecip_{t}")
                    nc.vector.reduce_sum(denom, U_nat, axis=AXX)
                    nc.vector.reciprocal(srecip, denom)
                    U_norm = hp.tile([128, 4, KR], F16, tag="unorm", bufs=3,
                                     name=f"unorm_{t}")
                    for ns in range(4):
                        nc.vector.tensor_scalar_mul(U_norm[:, ns, :],
                                                    U_nat[:, ns, :],
                                                    srecip[:, ns:ns + 1])
                    return U_norm

                def stage_T(t):
                    U_norm = state[t]["U_norm"]
                    UT_ps = psH.tile([128, 2, NSLAB], F16, tag="utps", bufs=2,
                                     name=f"utps_{t}")
                    for ns in range(4):
                        for kb in range(2):
                            mm(UT_ps[:, kb, ns * 128:(ns + 1) * 128],
                               lhsT=U_norm[:, ns, kb * 128:(kb + 1) * 128],
                               rhs=id16_sb, is_transpose=True,
                               start=(ns == 0 and kb == 0),
                               stop=(ns == 3 and kb == 1))
                    UT_sb = hp.tile([128, 2, NSLAB], F16, tag="utsb", bufs=2,
                                    name=f"utsb_{t}")
                    nc.vector.tensor_copy(UT_sb, UT_ps)
                    return UT_sb

                def stage_av(t):
                    s, h = t // 8, t % 8
                    hp_, ct_h = (h % 2) * 64, h // 2
                    UT_sb = state[t]["UT_sb"]
                    av_ps = psH.tile([128, NSLAB], F32, tag="av", bufs=2,
                                     name=f"av_{t}")
                    for krt in range(2):
                        mm(av_ps[hp_:hp_ + 64, :],
                           lhsT=vp_sb[:, krt, h * 64:(h + 1) * 64],
                           rhs=UT_sb[:, krt, :],
                           start=(krt == 0), stop=(krt == 1))
                    nc.scalar.copy(outU[hp_:hp_ + 64, ct_h, s % 2, :],
                                   av_ps[hp_:hp_ + 64, :])

                def stage_C(s, jc):
                    f_ps = psH.tile([128, NSLAB], F32, tag="fps", bufs=2,
                                    name=f"fps_{s}_{jc}")
                    for ct in range(4):
                        mm(f_ps, lhsT=wo_sb[:, ct, jc * 128:(jc + 1) * 128],
                           rhs=outU[:, ct, s % 2, :],
                           start=(ct == 0), stop=(ct == 3))
                    ot = hp.tile([128, NSLAB], F16, tag="ot", bufs=4,
                                 name=f"ot_{s}_{jc}")
                    nc.scalar.copy(ot, f_ps)
                    nc.sync.dma_start(
                        out=out_d[jc * 128:(jc + 1) * 128,
                                  s * NSLAB:(s + 1) * NSLAB], in_=ot)

                # C(s-1) chunk schedule: 8 jc chunks over steps h=3..7 of slab s
                c_sched = {3: [0, 1], 4: [2, 3], 5: [4], 6: [5], 7: [6, 7]}

                for t in range(NSTEP):
                    s, h = t // 8, t % 8
                    state[t] = {}
                    state[t]["U_norm"] = stage_nat(t)
                    if t - 2 >= 0:
                        state[t - 2]["UT_sb"] = stage_T(t - 2)
                    if t - 3 >= 0:
                        stage_av(t - 3)
                    if s >= 1:
                        for jc in c_sched.get(h, []):
                            stage_C(s - 1, jc)
                # epilogue
                for t in (NSTEP - 2, NSTEP - 1):
                    state[t]["UT_sb"] = stage_T(t)
                for t in (NSTEP - 3, NSTEP - 2, NSTEP - 1):
                    stage_av(t)
                for jc in range(8):
                    stage_C(SLABS - 1, jc)

    nc.compile()
    return nc


def kernel(x, W_qkv, E, W_out, b_out):
    x = np.ascontiguousarray(np.asarray(x, dtype=np.float32))
    W_qkv = np.asarray(W_qkv, dtype=np.float32)
    E_np = np.asarray(E, dtype=np.float32)
    W_out = np.asarray(W_out, dtype=np.float32)
    b_out = np.asarray(b_out, dtype=np.float32)

    if "nc" not in _cache:
        _cache["nc"] = build_program()
    nc = _cache["nc"]

    E_t = np.ascontiguousarray(E_np.reshape(NT, 128, KR))
    id32 = np.eye(128, dtype=np.float32)
    id16 = np.eye(128, dtype=np.float16)
    in_maps = []
    for core in range(8):
        b, g = core // 2, core % 2
        cols = slice(g * CG, (g + 1) * CG)
        xn_t = x[b].reshape(NT, 128, DIM)
        Wq_t = np.ascontiguousarray(
            (W_qkv[:, 0 * DIM:1 * DIM][:, cols] * SCALE)).reshape(DC, 128, CG)
        Wk_t = np.ascontiguousarray(W_qkv[:, 1 * DIM:2 * DIM][:, cols]).reshape(
            DC, 128, CG)
        Wv_t = np.ascontiguousarray(W_qkv[:, 2 * DIM:3 * DIM][:, cols]).reshape(
            DC, 128, CG)
        Wo_t = np.ascontiguousarray(
            W_out[g * CG:(g + 1) * CG, :].astype(np.float16)).reshape(
            CG // 128, 128, DIM)
        in_maps.append({
            "xn": xn_t, "E": E_t, "Wq": Wq_t, "Wk": Wk_t, "Wv": Wv_t,
            "Wo": Wo_t, "id32": id32, "id16": id16,
        })

    trace = bool(int(os.environ.get("KERNEL_TRACE", "0")))
    res = run_bass_kernel_spmd(nc, in_maps, core_ids=list(range(8)), trace=trace)
    _cache["last_results"] = res

    # partials come back transposed [DIM, SEQ] fp16; sum per batch in f32
    accT = np.zeros((4, DIM, SEQ), dtype=np.float32)
    for core in range(8):
        accT[core // 2] += res.results[core]["out"].astype(np.float32)
    out = np.ascontiguousarray(accT.transpose(0, 2, 1))
    out += b_out[None, None, :]
    return out


# revision 13
# speedup vs baseline: 2.5447x; 1.0353x over previous
"""Linformer self-attention on 8 Trainium2 NeuronCores.

Sharding: core = (batch b, head-group g) with b = core//2, g = core%2.
Each core computes attention for batch b and its 8 heads (512 of the 1024
channels), then a row-sharded W_out matmul producing a partial output in
transposed [1024, 4096] fp16 layout; the host sums the two partials per
batch in f32, transposes once, and adds b_out.

Key algebraic restructure vs the naive dataflow (Linformer associativity):
  k_proj = E^T (x Wk) = (E^T x) Wk   and likewise for v_proj.
Computing xE^T = x^T E first (shared by k and v) cuts the projection phase
from ~328k to ~82k PE cycles.

Per-core dataflow:
  FRONT: stream xn (natural [n,d] tiles) + E tiles; accumulate
    xET[d,kr] = sum_n x[n,d] E[n,kr] in PSUM; PE-transpose each xn tile to
    xnT for the q path; every 4 tiles emit one slab's qT = Wq^T x^T
    (Wq pre-scaled by 1/sqrt(hd) on host); then kpT = Wk^T xET and
    vp = xET^T-slices @ Wv.
  HEADS (per slab s, head h, software-pipelined): scores in natural
    layout [n,kr] via one PE pass; row max via free-axis reduce (negated);
    exp on ScalarE with per-partition bias; denominator via free-axis
    reduce_sum; tiny [128,4] reciprocal; normalize U (per-partition
    scalar mult); PE-transpose of normalized U (fp16); AV matmul;
    W_out chunks of the previous slab interleaved between heads.

Precision: q/k score chain fully f32r; U/vp/attn-out/W_out/output fp16.
"""

import os
import numpy as np

import concourse.bacc as bacc
import concourse.tile as tile
from concourse import mybir
from concourse.bass_utils import run_bass_kernel_spmd

F32 = mybir.dt.float32
F16 = mybir.dt.float16
F32R = mybir.dt.float32r
EXP = mybir.ActivationFunctionType.Exp
AXX = mybir.AxisListType.X

DIM, SEQ, KR, HD = 1024, 4096, 256, 64
CG = 512               # channels per head-group (8 heads x 64)
NSLAB = 512
SLABS = SEQ // NSLAB   # 8
NT = SEQ // 128        # 32 natural n-tiles
DC = DIM // 128        # 8 d-tiles
SCALE = HD ** -0.5

_cache = {}


def build_program():
    nc = bacc.Bacc("TRN2", target_bir_lowering=False, debug=False, num_devices=8)

    xn_d = nc.dram_tensor("xn", [NT, 128, DIM], F32R, kind="ExternalInput")
    Ed = nc.dram_tensor("E", [NT, 128, KR], F32R, kind="ExternalInput")
    Wq = nc.dram_tensor("Wq", [DC, 128, CG], F32R, kind="ExternalInput")
    Wk = nc.dram_tensor("Wk", [DC, 128, CG], F32R, kind="ExternalInput")
    Wv = nc.dram_tensor("Wv", [DC, 128, CG], F32R, kind="ExternalInput")
    Wo = nc.dram_tensor("Wo", [CG // 128, 128, DIM], F16, kind="ExternalInput")
    id32_d = nc.dram_tensor("id32", [128, 128], F32R, kind="ExternalInput")
    id16_d = nc.dram_tensor("id16", [128, 128], F16, kind="ExternalInput")
    out_d = nc.dram_tensor("out", [DIM, SEQ], F16, kind="ExternalOutput")
    dbg = os.environ.get("KERNEL_DEBUG", "0") == "1"
    if dbg:
        dbg_xET = nc.dram_tensor("dbg_xET", [128, DC, KR], F32R, kind="ExternalOutput")
        dbg_kpT = nc.dram_tensor("dbg_kpT", [128, 4, KR], F32R, kind="ExternalOutput")
        dbg_vp = nc.dram_tensor("dbg_vp", [128, 2, CG], F16, kind="ExternalOutput")
        dbg_qt = nc.dram_tensor("dbg_qt", [128, 4, NSLAB], F32R, kind="ExternalOutput")

    mm = nc.tensor.matmul

    with tile.TileContext(nc) as tc:
        with tc.tile_pool(name="const", bufs=1) as const:
            wq_sb = const.tile([128, DC, CG], F32R)
            wo_sb = const.tile([128, 4, DIM], F16)
            id32_sb = const.tile([128, 128], F32R)
            id16_sb = const.tile([128, 128], F16)
            xET_sb = const.tile([128, DC, KR], F32R)   # x^T E  [d, kr]
            kpT_sb = const.tile([128, 4, KR], F32R)    # (kp)^T [c, kr]
            vp_sb = const.tile([128, 2, CG], F16)      # vp     [kr, c]
            qt = const.tile([128, 4, SLABS, NSLAB], F32R)  # q^T, all slabs

            nc.sync.dma_start(out=id32_sb, in_=id32_d[:, :])
            nc.sync.dma_start(out=id16_sb, in_=id16_d[:, :])

            # ---------------- FRONT: xET accumulate + xnT transposes + qT ----
            with tc.tile_pool(name="frA", bufs=1) as frA, \
                 tc.tile_pool(name="psQ", bufs=1, space="PSUM") as psQ:
                wk_sb = frA.tile([128, DC, CG], F32R)
                wv_sb = frA.tile([128, DC, CG], F32R)

                def rr_copy(i, out, in_):
                    eng = (nc.vector.tensor_copy, nc.scalar.copy)[i % 2]
                    eng(out, in_)

                with tc.tile_pool(name="psE", bufs=1, space="PSUM") as psE:
                    xET_ps = psE.tile([128, DC, KR], F32)  # 4 banks
                    for nt in range(NT):
                        xt = frA.tile([128, DIM], F32R, tag="xn", bufs=3,
                                      name=f"xt_{nt}")
                        nc.sync.dma_start(out=xt, in_=xn_d[nt])
                        et = frA.tile([128, KR], F32R, tag="et", bufs=3,
                                      name=f"et_{nt}")
                        nc.sync.dma_start(out=et, in_=Ed[nt])
                        if nt == 0:
                            for dc in range(2):
                                nc.sync.dma_start(out=wq_sb[:, dc, :], in_=Wq[dc])
                        elif nt == 1:
                            for dc in range(2, DC):
                                nc.sync.dma_start(out=wq_sb[:, dc, :], in_=Wq[dc])
                        elif nt == 2:
                            for ct in range(4):
                                nc.sync.dma_start(out=wo_sb[:, ct, :], in_=Wo[ct])
                        elif nt == 3:
                            for dc in range(DC):
                                nc.sync.dma_start(out=wk_sb[:, dc, :], in_=Wk[dc])
                        elif nt == 4:
                            for dc in range(DC):
                                nc.sync.dma_start(out=wv_sb[:, dc, :], in_=Wv[dc])
                        for dsub in range(DC):
                            mm(xET_ps[:, dsub, :],
                               lhsT=xt[:, dsub * 128:(dsub + 1) * 128], rhs=et,
                               start=(nt == 0 and dsub % 2 == 0),
                               stop=(nt == NT - 1))
                        # transpose this n-tile for the q path
                        s, j = nt // 4, nt % 4
                        if j == 0:
                            xnT = frA.tile([128, DC, 4, 128], F32R, tag="xnT",
                                           bufs=2, name=f"xnT_{s}")
                        for half in range(2):
                            tp = psQ.tile([128, 4, 128], F32R, tag="tp", bufs=2,
                                          name=f"tp_{nt}_{half}")
                            for i in range(4):
                                dsub = half * 4 + i
                                mm(tp[:, i, :],
                                   lhsT=xt[:, dsub * 128:(dsub + 1) * 128],
                                   rhs=id32_sb, is_transpose=True,
                                   start=(i == 0), stop=(i == 3))
                            rr_copy(2 * nt + half,
                                    xnT[:, half * 4:(half + 1) * 4, j, :], tp)
                        if j == 3:
                            for ct in range(4):
                                q_ps = psQ.tile([128, NSLAB], F32, tag="qps",
                                                bufs=2, name=f"qps_{s}_{ct}")
                                for dc in range(DC):
                                    mm(q_ps,
                                       lhsT=wq_sb[:, dc, ct * 128:(ct + 1) * 128],
                                       rhs=xnT[:, dc, :, :],
                                       start=(dc == 0), stop=(dc == DC - 1))
                                rr_copy(ct, qt[:, ct, s, :], q_ps)
                    for d2 in range(0, DC, 2):
                        rr_copy(d2 // 2, xET_sb[:, d2:d2 + 2, :],
                                xET_ps[:, d2:d2 + 2, :])

                # kpT / vp from xET (reuses the 4 banks freed by psE)
                with tc.tile_pool(name="psKV", bufs=1, space="PSUM") as psKV:
                    kpT_ps = psKV.tile([128, 4, KR], F32)
                    for dc in range(DC):
                        for ct in range(4):
                            mm(kpT_ps[:, ct, :],
                               lhsT=wk_sb[:, dc, ct * 128:(ct + 1) * 128],
                               rhs=xET_sb[:, dc, :],
                               start=(dc == 0 and ct % 2 == 0),
                               stop=(dc == DC - 1))
                    nc.vector.tensor_copy(kpT_sb, kpT_ps)
                    vp_ps = psKV.tile([128, 2, CG], F32)
                    for dc in range(DC):
                        for krt in range(2):
                            mm(vp_ps[:, krt, :],
                               lhsT=xET_sb[:, dc, krt * 128:(krt + 1) * 128],
                               rhs=wv_sb[:, dc, :],
                               start=(dc == 0), stop=(dc == DC - 1))
                    nc.vector.tensor_copy(vp_sb, vp_ps)

            if dbg:
                nc.sync.dma_start(out=dbg_xET[:, :, :], in_=xET_sb)
                nc.sync.dma_start(out=dbg_kpT[:, :, :], in_=kpT_sb)
                nc.sync.dma_start(out=dbg_vp[:, :, :], in_=vp_sb)
                nc.sync.dma_start(out=dbg_qt[:, :, :], in_=qt[:, :, 0, :])

            # ---------------- HEADS epoch (software-pipelined) --------------
            with tc.tile_pool(name="hp", bufs=1) as hp, \
                 tc.tile_pool(name="psH", bufs=1, space="PSUM") as psH:
                outU = hp.tile([128, 4, 2, NSLAB], F16)  # 2-slab ring

                NSTEP = SLABS * 8
                state = {}

                def stage_nat(t):
                    s, h = t // 8, t % 8
                    hp_, ct_h = (h % 2) * 64, h // 2
                    kph = kpT_sb[hp_:hp_ + 64, ct_h, :]
                    nat = [None, None]
                    mrows = hp.tile([128, 4], F32, tag="mrows", bufs=3,
                                    name=f"mrows_{t}")
                    U_nat = hp.tile([128, 4, KR], F16, tag="unat", bufs=2,
                                    name=f"unat_{t}")
                    for hf in range(2):
                        natp = psH.tile([128, 2, KR], F32, tag="nat", bufs=2,
                                        name=f"nat_{t}_{hf}")
                        for i in range(2):
                            ns = hf * 2 + i
                            qh = qt[hp_:hp_ + 64, ct_h, s,
                                    ns * 128:(ns + 1) * 128]
                            mm(natp[:, i, :], lhsT=qh, rhs=kph,
                               start=(i == 0), stop=(i == 1))
                        nc.vector.reduce_max(mrows[:, 2 * hf:2 * hf + 2], natp,
                                             axis=AXX, negate=True)
                        for i in range(2):
                            ns = hf * 2 + i
                            nc.scalar.activation(U_nat[:, ns, :], natp[:, i, :],
                                                 EXP, bias=mrows[:, ns:ns + 1],
                                                 scale=1.0)
                        nat[hf] = natp
                    denom = hp.tile([128, 4], F32, tag="denom", bufs=2,
                                    name=f"denom_{t}")
                    srecip = hp.tile([128, 4], F32, tag="srecip", bufs=2,
                                     name=f"srecip_{t}")
                    nc.vector.reduce_sum(denom, U_nat, axis=AXX)
                    nc.vector.reciprocal(srecip, denom)
                    U_norm = hp.tile([128, 4, KR], F16, tag="unorm", bufs=3,
                                     name=f"unorm_{t}")
                    for ns in range(4):
                        nc.vector.tensor_scalar_mul(U_norm[:, ns, :],
                                                    U_nat[:, ns, :],
                                                    srecip[:, ns:ns + 1])
                    return U_norm

                def stage_T(t):
                    U_norm = state[t]["U_norm"]
                    UT_ps = psH.tile([128, 2, NSLAB], F16, tag="utps", bufs=2,
                                     name=f"utps_{t}")
                    for ns in range(4):
                        for kb in range(2):
                            mm(UT_ps[:, kb, ns * 128:(ns + 1) * 128],
                               lhsT=U_norm[:, ns, kb * 128:(kb + 1) * 128],
                               rhs=id16_sb, is_transpose=True,
                               start=(ns == 0 and kb == 0),
                               stop=(ns == 3 and kb == 1))
                    UT_sb = hp.tile([128, 2, NSLAB], F16, tag="utsb", bufs=2,
                                    name=f"utsb_{t}")
                    nc.vector.tensor_copy(UT_sb, UT_ps)
                    return UT_sb

                def stage_av(t):
                    s, h = t // 8, t % 8
                    hp_, ct_h = (h % 2) * 64, h // 2
                    UT_sb = state[t]["UT_sb"]
                    av_ps = psH.tile([128, NSLAB], F32, tag="av", bufs=2,
                                     name=f"av_{t}")
                    for krt in range(2):
                        mm(av_ps[hp_:hp_ + 64, :],
                           lhsT=vp_sb[:, krt, h * 64:(h + 1) * 64],
                           rhs=UT_sb[:, krt, :],
                           start=(krt == 0), stop=(krt == 1))
                    nc.scalar.copy(outU[hp_:hp_ + 64, ct_h, s % 2, :],
                                   av_ps[hp_:hp_ + 64, :])

                def stage_C(s, jc):
                    f_ps = psH.tile([128, NSLAB], F32, tag="fps", bufs=2,
                                    name=f"fps_{s}_{jc}")
                    for ct in range(4):
                        mm(f_ps, lhsT=wo_sb[:, ct, jc * 128:(jc + 1) * 128],
                           rhs=outU[:, ct, s % 2, :],
                           start=(ct == 0), stop=(ct == 3))
                    ot = hp.tile([128, NSLAB], F16, tag="ot", bufs=4,
                                 name=f"ot_{s}_{jc}")
                    nc.scalar.copy(ot, f_ps)
                    nc.sync.dma_start(
                        out=out_d[jc * 128:(jc + 1) * 128,
                                  s * NSLAB:(s + 1) * NSLAB], in_=ot)

                # C(s-1) chunk schedule: 8 jc chunks over steps h=3..7 of slab s
                c_sched = {3: [0, 1], 4: [2, 3], 5: [4], 6: [5], 7: [6, 7]}

                for t in range(NSTEP):
                    s, h = t // 8, t % 8
                    state[t] = {}
                    state[t]["U_norm"] = stage_nat(t)
                    if t - 2 >= 0:
                        state[t - 2]["UT_sb"] = stage_T(t - 2)
                    if t - 3 >= 0:
                        stage_av(t - 3)
                    if s >= 1:
                        for jc in c_sched.get(h, []):
                            stage_C(s - 1, jc)
                # epilogue
                for t in (NSTEP - 2, NSTEP - 1):
                    state[t]["UT_sb"] = stage_T(t)
                for t in (NSTEP - 3, NSTEP - 2, NSTEP - 1):
                    stage_av(t)
                for jc in range(8):
                    stage_C(SLABS - 1, jc)

    nc.compile()
    return nc


def kernel(x, W_qkv, E, W_out, b_out):
    x = np.ascontiguousarray(np.asarray(x, dtype=np.float32))
    W_qkv = np.asarray(W_qkv, dtype=np.float32)
    E_np = np.asarray(E, dtype=np.float32)
    W_out = np.asarray(W_out, dtype=np.float32)
    b_out = np.asarray(b_out, dtype=np.float32)

    if "nc" not in _cache:
        _cache["nc"] = build_program()
    nc = _cache["nc"]

    E_t = np.ascontiguousarray(E_np.reshape(NT, 128, KR))
    id32 = np.eye(128, dtype=np.float32)
    id16 = np.eye(128, dtype=np.float16)
    in_maps = []
    for core in range(8):
        b, g = core // 2, core % 2
        cols = slice(g * CG, (g + 1) * CG)
        xn_t = x[b].reshape(NT, 128, DIM)
        Wq_t = np.ascontiguousarray(
            (W_qkv[:, 0 * DIM:1 * DIM][:, cols] * SCALE)).reshape(DC, 128, CG)
        Wk_t = np.ascontiguousarray(W_qkv[:, 1 * DIM:2 * DIM][:, cols]).reshape(
            DC, 128, CG)
        Wv_t = np.ascontiguousarray(W_qkv[:, 2 * DIM:3 * DIM][:, cols]).reshape(
            DC, 128, CG)
        Wo_t = np.ascontiguousarray(
            W_out[g * CG:(g + 1) * CG, :].astype(np.float16)).reshape(
            CG // 128, 128, DIM)
        in_maps.append({
            "xn": xn_t, "E": E_t, "Wq": Wq_t, "Wk": Wk_t, "Wv": Wv_t,
            "Wo": Wo_t, "id32": id32, "id16": id16,
        })

    trace = bool(int(os.environ.get("KERNEL_TRACE", "0")))
    res = run_bass_kernel_spmd(nc, in_maps, core_ids=list(range(8)), trace=trace)
    _cache["last_results"] = res

    # partials come back transposed [DIM, SEQ] fp16; sum per batch in f32
    accT = np.zeros((4, DIM, SEQ), dtype=np.float32)
    for core in range(8):
        accT[core // 2] += res.results[core]["out"].astype(np.float32)
    out = np.ascontiguousarray(accT.transpose(0, 2, 1))
    out += b_out[None, None, :]
    return out


# revision 14
# speedup vs baseline: 2.6814x; 1.0537x over previous
"""Linformer self-attention on 8 Trainium2 NeuronCores.

Sharding: core = (batch b, head-group g) with b = core//2, g = core%2.
Each core computes attention for batch b and its 8 heads (512 of the 1024
channels), then a row-sharded W_out matmul producing a partial output in
transposed [1024, 4096] fp16 layout; the host sums the two partials per
batch in f32, transposes once, and adds b_out.

Key algebraic restructure vs the naive dataflow (Linformer associativity):
  k_proj = E^T (x Wk) = (E^T x) Wk   and likewise for v_proj.
Computing xE^T = x^T E first (shared by k and v) cuts the projection phase
from ~328k to ~82k PE cycles.

Per-core dataflow:
  FRONT: stream xn (natural [n,d] tiles) + E tiles; accumulate
    xET[d,kr] = sum_n x[n,d] E[n,kr] in PSUM; PE-transpose each xn tile to
    xnT for the q path; every 4 tiles emit one slab's qT = Wq^T x^T
    (Wq pre-scaled by 1/sqrt(hd) on host); then kpT = Wk^T xET and
    vp = xET^T-slices @ Wv.
  HEADS (per slab s, head h, software-pipelined): scores in natural
    layout [n,kr] via one PE pass; row max via free-axis reduce (negated);
    exp on ScalarE with per-partition bias; denominator via free-axis
    reduce_sum; tiny [128,4] reciprocal; normalize U (per-partition
    scalar mult); PE-transpose of normalized U (fp16); AV matmul;
    W_out chunks of the previous slab interleaved between heads.

Precision: q/k score chain fully f32r; U/vp/attn-out/W_out/output fp16.
"""

import os
import numpy as np

import concourse.bacc as bacc
import concourse.tile as tile
from concourse import mybir
from concourse.bass_utils import run_bass_kernel_spmd

F32 = mybir.dt.float32
F16 = mybir.dt.float16
F32R = mybir.dt.float32r
EXP = mybir.ActivationFunctionType.Exp
AXX = mybir.AxisListType.X

DIM, SEQ, KR, HD = 1024, 4096, 256, 64
CG = 512               # channels per head-group (8 heads x 64)
NSLAB = 512
SLABS = SEQ // NSLAB   # 8
NT = SEQ // 128        # 32 natural n-tiles
DC = DIM // 128        # 8 d-tiles
SCALE = HD ** -0.5

_cache = {}


def build_program():
    nc = bacc.Bacc("TRN2", target_bir_lowering=False, debug=False, num_devices=8)

    xn_d = nc.dram_tensor("xn", [NT, 128, DIM], F32R, kind="ExternalInput")
    xT_d = nc.dram_tensor("xT", [DC, 128, SEQ], F32R, kind="ExternalInput")
    Ed = nc.dram_tensor("E", [NT, 128, KR], F32R, kind="ExternalInput")
    Wq = nc.dram_tensor("Wq", [DC, 128, CG], F32R, kind="ExternalInput")
    Wk = nc.dram_tensor("Wk", [DC, 128, CG], F32R, kind="ExternalInput")
    Wv = nc.dram_tensor("Wv", [DC, 128, CG], F32R, kind="ExternalInput")
    Wo = nc.dram_tensor("Wo", [CG // 128, 128, DIM], F16, kind="ExternalInput")
    id16_d = nc.dram_tensor("id16", [128, 128], F16, kind="ExternalInput")
    out_d = nc.dram_tensor("out", [DIM, SEQ], F16, kind="ExternalOutput")
    dbg = os.environ.get("KERNEL_DEBUG", "0") == "1"
    if dbg:
        dbg_xET = nc.dram_tensor("dbg_xET", [128, DC, KR], F32R, kind="ExternalOutput")
        dbg_kpT = nc.dram_tensor("dbg_kpT", [128, 4, KR], F32R, kind="ExternalOutput")
        dbg_vp = nc.dram_tensor("dbg_vp", [128, 2, CG], F16, kind="ExternalOutput")
        dbg_qt = nc.dram_tensor("dbg_qt", [128, 4, NSLAB], F32R, kind="ExternalOutput")

    mm = nc.tensor.matmul

    with tile.TileContext(nc) as tc:
        with tc.tile_pool(name="const", bufs=1) as const:
            wq_sb = const.tile([128, DC, CG], F32R)
            wo_sb = const.tile([128, 4, DIM], F16)
            id16_sb = const.tile([128, 128], F16)
            xET_sb = const.tile([128, DC, KR], F32R)   # x^T E  [d, kr]
            kpT_sb = const.tile([128, 4, KR], F32R)    # (kp)^T [c, kr]
            vp_sb = const.tile([128, 2, CG], F16)      # vp     [kr, c]
            qt = const.tile([128, 4, SLABS, NSLAB], F32R)  # q^T, all slabs

            nc.sync.dma_start(out=id16_sb, in_=id16_d[:, :])

            # ---------------- FRONT: xET accumulate + xnT transposes + qT ----
            with tc.tile_pool(name="frA", bufs=1) as frA, \
                 tc.tile_pool(name="psQ", bufs=1, space="PSUM") as psQ:
                wk_sb = frA.tile([128, DC, CG], F32R)
                wv_sb = frA.tile([128, DC, CG], F32R)

                def rr_copy(i, out, in_):
                    eng = (nc.vector.tensor_copy, nc.scalar.copy)[i % 2]
                    eng(out, in_)

                with tc.tile_pool(name="psE", bufs=1, space="PSUM") as psE:
                    xET_ps = psE.tile([128, DC, KR], F32)  # 4 banks
                    for nt in range(NT):
                        xt = frA.tile([128, DIM], F32R, tag="xn", bufs=3,
                                      name=f"xt_{nt}")
                        nc.sync.dma_start(out=xt, in_=xn_d[nt])
                        et = frA.tile([128, KR], F32R, tag="et", bufs=3,
                                      name=f"et_{nt}")
                        nc.sync.dma_start(out=et, in_=Ed[nt])
                        if nt == 0:
                            for dc in range(2):
                                nc.sync.dma_start(out=wq_sb[:, dc, :], in_=Wq[dc])
                        elif nt == 1:
                            for dc in range(2, DC):
                                nc.sync.dma_start(out=wq_sb[:, dc, :], in_=Wq[dc])
                        elif nt == 2:
                            for ct in range(4):
                                nc.sync.dma_start(out=wo_sb[:, ct, :], in_=Wo[ct])
                        elif nt == 3:
                            for dc in range(DC):
                                nc.sync.dma_start(out=wk_sb[:, dc, :], in_=Wk[dc])
                        elif nt == 4:
                            for dc in range(DC):
                                nc.sync.dma_start(out=wv_sb[:, dc, :], in_=Wv[dc])
                        for dsub in range(DC):
                            mm(xET_ps[:, dsub, :],
                               lhsT=xt[:, dsub * 128:(dsub + 1) * 128], rhs=et,
                               start=(nt == 0 and dsub % 2 == 0),
                               stop=(nt == NT - 1))
                        # q path: consume a host-transposed xT slab every 4 tiles
                        s, j = nt // 4, nt % 4
                        if j == 0:
                            xs = frA.tile([128, DC, NSLAB], F32R, tag="xs",
                                          bufs=2, name=f"xs_{s}")
                            for dc in range(DC):
                                nc.sync.dma_start(
                                    out=xs[:, dc, :],
                                    in_=xT_d[dc, :, s * NSLAB:(s + 1) * NSLAB])
                        if j == 3:
                            for ct in range(4):
                                q_ps = psQ.tile([128, NSLAB], F32, tag="qps",
                                                bufs=2, name=f"qps_{s}_{ct}")
                                for dc in range(DC):
                                    mm(q_ps,
                                       lhsT=wq_sb[:, dc, ct * 128:(ct + 1) * 128],
                                       rhs=xs[:, dc, :],
                                       start=(dc == 0), stop=(dc == DC - 1))
                                rr_copy(ct, qt[:, ct, s, :], q_ps)
                    for d2 in range(0, DC, 2):
                        rr_copy(d2 // 2, xET_sb[:, d2:d2 + 2, :],
                                xET_ps[:, d2:d2 + 2, :])

                # kpT / vp from xET (reuses the 4 banks freed by psE)
                with tc.tile_pool(name="psKV", bufs=1, space="PSUM") as psKV:
                    kpT_ps = psKV.tile([128, 4, KR], F32)
                    for dc in range(DC):
                        for ct in range(4):
                            mm(kpT_ps[:, ct, :],
                               lhsT=wk_sb[:, dc, ct * 128:(ct + 1) * 128],
                               rhs=xET_sb[:, dc, :],
                               start=(dc == 0 and ct % 2 == 0),
                               stop=(dc == DC - 1))
                    nc.vector.tensor_copy(kpT_sb, kpT_ps)
                    vp_ps = psKV.tile([128, 2, CG], F32)
                    for dc in range(DC):
                        for krt in range(2):
                            mm(vp_ps[:, krt, :],
                               lhsT=xET_sb[:, dc, krt * 128:(krt + 1) * 128],
                               rhs=wv_sb[:, dc, :],
                               start=(dc == 0), stop=(dc == DC - 1))
                    nc.vector.tensor_copy(vp_sb, vp_ps)

            if dbg:
                nc.sync.dma_start(out=dbg_xET[:, :, :], in_=xET_sb)
                nc.sync.dma_start(out=dbg_kpT[:, :, :], in_=kpT_sb)
                nc.sync.dma_start(out=dbg_vp[:, :, :], in_=vp_sb)
                nc.sync.dma_start(out=dbg_qt[:, :, :], in_=qt[:, :, 0, :])

            # ---------------- HEADS epoch (software-pipelined) --------------
            with tc.tile_pool(name="hp", bufs=1) as hp, \
                 tc.tile_pool(name="psH", bufs=1, space="PSUM") as psH:
                outU = hp.tile([128, 4, 2, NSLAB], F16)  # 2-slab ring

                NSTEP = SLABS * 8
                state = {}

                def stage_nat(t):
                    s, h = t // 8, t % 8
                    hp_, ct_h = (h % 2) * 64, h // 2
                    kph = kpT_sb[hp_:hp_ + 64, ct_h, :]
                    nat = [None, None]
                    mrows = hp.tile([128, 4], F32, tag="mrows", bufs=3,
                                    name=f"mrows_{t}")
                    U_nat = hp.tile([128, 4, KR], F16, tag="unat", bufs=2,
                                    name=f"unat_{t}")
                    for hf in range(2):
                        natp = psH.tile([128, 2, KR], F32, tag="nat", bufs=2,
                                        name=f"nat_{t}_{hf}")
                        for i in range(2):
                            ns = hf * 2 + i
                            qh = qt[hp_:hp_ + 64, ct_h, s,
                                    ns * 128:(ns + 1) * 128]
                            mm(natp[:, i, :], lhsT=qh, rhs=kph,
                               start=(i == 0), stop=(i == 1))
                        nc.vector.reduce_max(mrows[:, 2 * hf:2 * hf + 2], natp,
                                             axis=AXX, negate=True)
                        for i in range(2):
                            ns = hf * 2 + i
                            nc.scalar.activation(U_nat[:, ns, :], natp[:, i, :],
                                                 EXP, bias=mrows[:, ns:ns + 1],
                                                 scale=1.0)
                        nat[hf] = natp
                    denom = hp.tile([128, 4], F32, tag="denom", bufs=2,
                                    name=f"denom_{t}")
                    srecip = hp.tile([128, 4], F32, tag="srecip", bufs=2,
                                     name=f"srecip_{t}")
                    nc.vector.reduce_sum(denom, U_nat, axis=AXX)
                    nc.vector.reciprocal(srecip, denom)
                    U_norm = hp.tile([128, 4, KR], F16, tag="unorm", bufs=3,
                                     name=f"unorm_{t}")
                    for ns in range(4):
                        nc.vector.tensor_scalar_mul(U_norm[:, ns, :],
                                                    U_nat[:, ns, :],
                                                    srecip[:, ns:ns + 1])
                    return U_norm

                def stage_T(t):
                    U_norm = state[t]["U_norm"]
                    UT_ps = psH.tile([128, 2, NSLAB], F16, tag="utps", bufs=2,
                                     name=f"utps_{t}")
                    for ns in range(4):
                        for kb in range(2):
                            mm(UT_ps[:, kb, ns * 128:(ns + 1) * 128],
                               lhsT=U_norm[:, ns, kb * 128:(kb + 1) * 128],
                               rhs=id16_sb, is_transpose=True,
                               start=(ns == 0 and kb == 0),
                               stop=(ns == 3 and kb == 1))
                    UT_sb = hp.tile([128, 2, NSLAB], F16, tag="utsb", bufs=2,
                                    name=f"utsb_{t}")
                    if t % 2 == 0:
                        nc.vector.tensor_copy(UT_sb, UT_ps)
                    else:
                        nc.scalar.copy(UT_sb, UT_ps)
                    return UT_sb

                def stage_av(t):
                    s, h = t // 8, t % 8
                    hp_, ct_h = (h % 2) * 64, h // 2
                    UT_sb = state[t]["UT_sb"]
                    av_ps = psH.tile([128, NSLAB], F32, tag="av", bufs=2,
                                     name=f"av_{t}")
                    for krt in range(2):
                        mm(av_ps[hp_:hp_ + 64, :],
                           lhsT=vp_sb[:, krt, h * 64:(h + 1) * 64],
                           rhs=UT_sb[:, krt, :],
                           start=(krt == 0), stop=(krt == 1))
                    nc.scalar.copy(outU[hp_:hp_ + 64, ct_h, s % 2, :],
                                   av_ps[hp_:hp_ + 64, :])

                def stage_C(s, jc):
                    f_ps = psH.tile([128, NSLAB], F32, tag="fps", bufs=2,
                                    name=f"fps_{s}_{jc}")
                    for ct in range(4):
                        mm(f_ps, lhsT=wo_sb[:, ct, jc * 128:(jc + 1) * 128],
                           rhs=outU[:, ct, s % 2, :],
                           start=(ct == 0), stop=(ct == 3))
                    ot = hp.tile([128, NSLAB], F16, tag="ot", bufs=4,
                                 name=f"ot_{s}_{jc}")
                    nc.scalar.copy(ot, f_ps)
                    nc.sync.dma_start(
                        out=out_d[jc * 128:(jc + 1) * 128,
                                  s * NSLAB:(s + 1) * NSLAB], in_=ot)

                # C(s-1) chunk schedule: 8 jc chunks over steps h=3..7 of slab s
                c_sched = {3: [0, 1], 4: [2, 3], 5: [4], 6: [5], 7: [6, 7]}

                for t in range(NSTEP):
                    s, h = t // 8, t % 8
                    state[t] = {}
                    state[t]["U_norm"] = stage_nat(t)
                    if t - 2 >= 0:
                        state[t - 2]["UT_sb"] = stage_T(t - 2)
                    if t - 3 >= 0:
                        stage_av(t - 3)
                    if s >= 1:
                        for jc in c_sched.get(h, []):
                            stage_C(s - 1, jc)
                # epilogue
                for t in (NSTEP - 2, NSTEP - 1):
                    state[t]["UT_sb"] = stage_T(t)
                for t in (NSTEP - 3, NSTEP - 2, NSTEP - 1):
                    stage_av(t)
                for jc in range(8):
                    stage_C(SLABS - 1, jc)

    nc.compile()
    return nc


def kernel(x, W_qkv, E, W_out, b_out):
    x = np.ascontiguousarray(np.asarray(x, dtype=np.float32))
    W_qkv = np.asarray(W_qkv, dtype=np.float32)
    E_np = np.asarray(E, dtype=np.float32)
    W_out = np.asarray(W_out, dtype=np.float32)
    b_out = np.asarray(b_out, dtype=np.float32)

    if "nc" not in _cache:
        _cache["nc"] = build_program()
    nc = _cache["nc"]

    E_t = np.ascontiguousarray(E_np.reshape(NT, 128, KR))
    id16 = np.eye(128, dtype=np.float16)
    in_maps = []
    for core in range(8):
        b, g = core // 2, core % 2
        cols = slice(g * CG, (g + 1) * CG)
        xn_t = x[b].reshape(NT, 128, DIM)
        xT_t = np.ascontiguousarray(x[b].T).reshape(DC, 128, SEQ)
        Wq_t = np.ascontiguousarray(
            (W_qkv[:, 0 * DIM:1 * DIM][:, cols] * SCALE)).reshape(DC, 128, CG)
        Wk_t = np.ascontiguousarray(W_qkv[:, 1 * DIM:2 * DIM][:, cols]).reshape(
            DC, 128, CG)
        Wv_t = np.ascontiguousarray(W_qkv[:, 2 * DIM:3 * DIM][:, cols]).reshape(
            DC, 128, CG)
        Wo_t = np.ascontiguousarray(
            W_out[g * CG:(g + 1) * CG, :].astype(np.float16)).reshape(
            CG // 128, 128, DIM)
        in_maps.append({
            "xn": xn_t, "xT": xT_t, "E": E_t, "Wq": Wq_t, "Wk": Wk_t,
            "Wv": Wv_t, "Wo": Wo_t, "id16": id16,
        })

    trace = bool(int(os.environ.get("KERNEL_TRACE", "0")))
    res = run_bass_kernel_spmd(nc, in_maps, core_ids=list(range(8)), trace=trace)
    _cache["last_results"] = res

    # partials come back transposed [DIM, SEQ] fp16; sum per batch in f32
    accT = np.zeros((4, DIM, SEQ), dtype=np.float32)
    for core in range(8):
        accT[core // 2] += res.results[core]["out"].astype(np.float32)
    out = np.ascontiguousarray(accT.transpose(0, 2, 1))
    out += b_out[None, None, :]
    return out


# revision 16
# speedup vs baseline: 2.7629x; 1.0304x over previous
"""Linformer self-attention on 8 Trainium2 NeuronCores.

Sharding: core = (batch b, head-group g) with b = core//2, g = core%2.
Each core computes attention for batch b and its 8 heads (512 of the 1024
channels), then a row-sharded W_out matmul producing a partial output in
transposed [1024, 4096] fp16 layout; the host sums the two partials per
batch in f32, transposes once, and adds b_out.

Key algebraic restructure vs the naive dataflow (Linformer associativity):
  k_proj = E^T (x Wk) = (E^T x) Wk   and likewise for v_proj.
Computing xE^T = x^T E first (shared by k and v) cuts the projection phase
from ~328k to ~82k PE cycles.

Per-core dataflow:
  FRONT: stream xn (natural [n,d] tiles) + E tiles; accumulate
    xET[d,kr] = sum_n x[n,d] E[n,kr] in PSUM; PE-transpose each xn tile to
    xnT for the q path; every 4 tiles emit one slab's qT = Wq^T x^T
    (Wq pre-scaled by 1/sqrt(hd) on host); then kpT = Wk^T xET and
    vp = xET^T-slices @ Wv.
  HEADS (per slab s, head h, software-pipelined): scores in natural
    layout [n,kr] via one PE pass; row max via free-axis reduce (negated);
    exp on ScalarE with per-partition bias; denominator via free-axis
    reduce_sum; tiny [128,4] reciprocal; normalize U (per-partition
    scalar mult); PE-transpose of normalized U (fp16); AV matmul;
    W_out chunks of the previous slab interleaved between heads.

Precision: q/k score chain fully f32r; U/vp/attn-out/W_out/output fp16.
"""

import os
import numpy as np

import concourse.bacc as bacc
import concourse.tile as tile
from concourse import mybir
from concourse.bass_utils import run_bass_kernel_spmd

F32 = mybir.dt.float32
F16 = mybir.dt.float16
F32R = mybir.dt.float32r
EXP = mybir.ActivationFunctionType.Exp
AXX = mybir.AxisListType.X

DIM, SEQ, KR, HD = 1024, 4096, 256, 64
CG = 512               # channels per head-group (8 heads x 64)
NSLAB = 512
SLABS = SEQ // NSLAB   # 8
NT = SEQ // 128        # 32 natural n-tiles
DC = DIM // 128        # 8 d-tiles
SCALE = HD ** -0.5

_cache = {}


def build_program():
    nc = bacc.Bacc("TRN2", target_bir_lowering=False, debug=False, num_devices=8)

    xn_d = nc.dram_tensor("xn", [NT, 128, DIM], F32R, kind="ExternalInput")
    xT_d = nc.dram_tensor("xT", [DC, 128, SEQ], F32R, kind="ExternalInput")
    Ed = nc.dram_tensor("E", [NT, 128, KR], F32R, kind="ExternalInput")
    Wq = nc.dram_tensor("Wq", [DC, 128, CG], F32R, kind="ExternalInput")
    Wk = nc.dram_tensor("Wk", [DC, 128, CG], F32R, kind="ExternalInput")
    Wv = nc.dram_tensor("Wv", [DC, 128, CG], F32R, kind="ExternalInput")
    Wo = nc.dram_tensor("Wo", [CG // 128, 128, DIM], F16, kind="ExternalInput")
    id16_d = nc.dram_tensor("id16", [128, 128], F16, kind="ExternalInput")
    out_d = nc.dram_tensor("out", [DIM, SEQ], F16, kind="ExternalOutput")
    dbg = os.environ.get("KERNEL_DEBUG", "0") == "1"
    if dbg:
        dbg_xET = nc.dram_tensor("dbg_xET", [128, DC, KR], F32R, kind="ExternalOutput")
        dbg_kpT = nc.dram_tensor("dbg_kpT", [128, 4, KR], F32R, kind="ExternalOutput")
        dbg_vp = nc.dram_tensor("dbg_vp", [128, 2, CG], F16, kind="ExternalOutput")
        dbg_qt = nc.dram_tensor("dbg_qt", [128, 4, NSLAB], F32R, kind="ExternalOutput")

    mm = nc.tensor.matmul

    with tile.TileContext(nc) as tc:
        with tc.tile_pool(name="const", bufs=1) as const:
            wq_sb = const.tile([128, DC, CG], F32R)
            wo_sb = const.tile([128, 4, DIM], F16)
            id16_sb = const.tile([128, 128], F16)
            xET_sb = const.tile([128, DC, KR], F32R)   # x^T E  [d, kr]
            kpT_sb = const.tile([128, 4, KR], F32R)    # (kp)^T [c, kr]
            vp_sb = const.tile([128, 2, CG], F16)      # vp     [kr, c]
            qt = const.tile([128, 4, SLABS, NSLAB], F32R)  # q^T, all slabs

            nc.sync.dma_start(out=id16_sb, in_=id16_d[:, :])

            # ---------------- FRONT: xET accumulate + xnT transposes + qT ----
            with tc.tile_pool(name="frA", bufs=1) as frA, \
                 tc.tile_pool(name="psQ", bufs=1, space="PSUM") as psQ:
                wk_sb = frA.tile([128, DC, CG], F32R)
                wv_sb = frA.tile([128, DC, CG], F32R)

                def rr_copy(i, out, in_):
                    eng = (nc.vector.tensor_copy, nc.scalar.copy)[i % 2]
                    eng(out, in_)

                with tc.tile_pool(name="psE", bufs=1, space="PSUM") as psE:
                    xET_ps = psE.tile([128, DC, KR], F32)  # 4 banks
                    for nt in range(NT):
                        xt = frA.tile([128, DIM], F32R, tag="xn", bufs=3,
                                      name=f"xt_{nt}")
                        nc.sync.dma_start(out=xt, in_=xn_d[nt])
                        et = frA.tile([128, KR], F32R, tag="et", bufs=3,
                                      name=f"et_{nt}")
                        nc.sync.dma_start(out=et, in_=Ed[nt])
                        if nt == 0:
                            for dc in range(2):
                                nc.sync.dma_start(out=wq_sb[:, dc, :], in_=Wq[dc])
                        elif nt == 1:
                            for dc in range(2, DC):
                                nc.sync.dma_start(out=wq_sb[:, dc, :], in_=Wq[dc])
                        elif nt == 2:
                            for ct in range(4):
                                nc.sync.dma_start(out=wo_sb[:, ct, :], in_=Wo[ct])
                        elif nt == 3:
                            for dc in range(DC):
                                nc.sync.dma_start(out=wk_sb[:, dc, :], in_=Wk[dc])
                        elif nt == 4:
                            for dc in range(DC):
                                nc.sync.dma_start(out=wv_sb[:, dc, :], in_=Wv[dc])
                        for dsub in range(DC):
                            mm(xET_ps[:, dsub, :],
                               lhsT=xt[:, dsub * 128:(dsub + 1) * 128], rhs=et,
                               start=(nt == 0 and dsub % 2 == 0),
                               stop=(nt == NT - 1))
                    for d2 in range(0, DC, 2):
                        rr_copy(d2 // 2, xET_sb[:, d2:d2 + 2, :],
                                xET_ps[:, d2:d2 + 2, :])

                # qT for the first two slabs (rest streams during heads)
                for s in range(2):
                    xs = frA.tile([128, DC, NSLAB], F32R, tag="xs",
                                  bufs=2, name=f"xs_{s}")
                    for dc in range(DC):
                        nc.sync.dma_start(
                            out=xs[:, dc, :],
                            in_=xT_d[dc, :, s * NSLAB:(s + 1) * NSLAB])
                    for ct in range(4):
                        q_ps = psQ.tile([128, NSLAB], F32, tag="qps",
                                        bufs=2, name=f"qps_{s}_{ct}")
                        for dc in range(DC):
                            mm(q_ps,
                               lhsT=wq_sb[:, dc, ct * 128:(ct + 1) * 128],
                               rhs=xs[:, dc, :],
                               start=(dc == 0), stop=(dc == DC - 1))
                        rr_copy(ct, qt[:, ct, s, :], q_ps)

                # kpT / vp from xET (reuses the 4 banks freed by psE)
                with tc.tile_pool(name="psKV", bufs=1, space="PSUM") as psKV:
                    kpT_ps = psKV.tile([128, 4, KR], F32)
                    for dc in range(DC):
                        for ct in range(4):
                            mm(kpT_ps[:, ct, :],
                               lhsT=wk_sb[:, dc, ct * 128:(ct + 1) * 128],
                               rhs=xET_sb[:, dc, :],
                               start=(dc == 0 and ct % 2 == 0),
                               stop=(dc == DC - 1))
                    nc.vector.tensor_copy(kpT_sb, kpT_ps)
                    vp_ps = psKV.tile([128, 2, CG], F32)
                    for dc in range(DC):
                        for krt in range(2):
                            mm(vp_ps[:, krt, :],
                               lhsT=xET_sb[:, dc, krt * 128:(krt + 1) * 128],
                               rhs=wv_sb[:, dc, :],
                               start=(dc == 0), stop=(dc == DC - 1))
                    nc.vector.tensor_copy(vp_sb, vp_ps)

            if dbg:
                nc.sync.dma_start(out=dbg_xET[:, :, :], in_=xET_sb)
                nc.sync.dma_start(out=dbg_kpT[:, :, :], in_=kpT_sb)
                nc.sync.dma_start(out=dbg_vp[:, :, :], in_=vp_sb)
                nc.sync.dma_start(out=dbg_qt[:, :, :], in_=qt[:, :, 0, :])

            # ---------------- HEADS epoch (software-pipelined) --------------
            with tc.tile_pool(name="hp", bufs=1) as hp, \
                 tc.tile_pool(name="psH", bufs=1, space="PSUM") as psH:
                outU = hp.tile([128, 4, 2, NSLAB], F16)  # 2-slab ring

                NSTEP = SLABS * 8
                state = {}

                def stage_nat(t):
                    s, h = t // 8, t % 8
                    hp_, ct_h = (h % 2) * 64, h // 2
                    kph = kpT_sb[hp_:hp_ + 64, ct_h, :]
                    nat = [None, None]
                    mrows = hp.tile([128, 4], F32, tag="mrows", bufs=3,
                                    name=f"mrows_{t}")
                    U_nat = hp.tile([128, 4, KR], F16, tag="unat", bufs=2,
                                    name=f"unat_{t}")
                    for hf in range(2):
                        natp = psH.tile([128, 2, KR], F32, tag="nat", bufs=2,
                                        name=f"nat_{t}_{hf}")
                        for i in range(2):
                            ns = hf * 2 + i
                            qh = qt[hp_:hp_ + 64, ct_h, s,
                                    ns * 128:(ns + 1) * 128]
                            mm(natp[:, i, :], lhsT=qh, rhs=kph,
                               start=(i == 0), stop=(i == 1))
                        nc.vector.reduce_max(mrows[:, 2 * hf:2 * hf + 2], natp,
                                             axis=AXX, negate=True)
                        for i in range(2):
                            ns = hf * 2 + i
                            nc.scalar.activation(U_nat[:, ns, :], natp[:, i, :],
                                                 EXP, bias=mrows[:, ns:ns + 1],
                                                 scale=1.0)
                        nat[hf] = natp
                    denom = hp.tile([128, 4], F16, tag="denom", bufs=2,
                                    name=f"denom_{t}")
                    srecip = hp.tile([128, 4], F32, tag="srecip", bufs=2,
                                     name=f"srecip_{t}")
                    with nc.allow_low_precision(
                            reason="softmax denom in [1,256]; fp16 ok"):
                        nc.vector.reduce_sum(denom, U_nat, axis=AXX)
                        nc.vector.reciprocal(srecip, denom)
                    U_norm = hp.tile([128, 4, KR], F16, tag="unorm", bufs=3,
                                     name=f"unorm_{t}")
                    for ns in range(4):
                        nc.vector.tensor_scalar_mul(U_norm[:, ns, :],
                                                    U_nat[:, ns, :],
                                                    srecip[:, ns:ns + 1])
                    return U_norm

                def stage_T(t):
                    U_norm = state[t]["U_norm"]
                    UT_ps = psH.tile([128, 2, NSLAB], F16, tag="utps", bufs=2,
                                     name=f"utps_{t}")
                    for ns in range(4):
                        for kb in range(2):
                            mm(UT_ps[:, kb, ns * 128:(ns + 1) * 128],
                               lhsT=U_norm[:, ns, kb * 128:(kb + 1) * 128],
                               rhs=id16_sb, is_transpose=True,
                               start=(ns == 0 and kb == 0),
                               stop=(ns == 3 and kb == 1))
                    UT_sb = hp.tile([128, 2, NSLAB], F16, tag="utsb", bufs=2,
                                    name=f"utsb_{t}")
                    if t % 2 == 0:
                        nc.vector.tensor_copy(UT_sb, UT_ps)
                    else:
                        nc.scalar.copy(UT_sb, UT_ps)
                    return UT_sb

                def stage_av(t):
                    s, h = t // 8, t % 8
                    hp_, ct_h = (h % 2) * 64, h // 2
                    UT_sb = state[t]["UT_sb"]
                    av_ps = psH.tile([128, NSLAB], F32, tag="av", bufs=2,
                                     name=f"av_{t}")
                    for krt in range(2):
                        mm(av_ps[hp_:hp_ + 64, :],
                           lhsT=vp_sb[:, krt, h * 64:(h + 1) * 64],
                           rhs=UT_sb[:, krt, :],
                           start=(krt == 0), stop=(krt == 1))
                    nc.scalar.copy(outU[hp_:hp_ + 64, ct_h, s % 2, :],
                                   av_ps[hp_:hp_ + 64, :])

                def stage_C(s, jc):
                    f_ps = psH.tile([128, NSLAB], F32, tag="fps", bufs=2,
                                    name=f"fps_{s}_{jc}")
                    for ct in range(4):
                        mm(f_ps, lhsT=wo_sb[:, ct, jc * 128:(jc + 1) * 128],
                           rhs=outU[:, ct, s % 2, :],
                           start=(ct == 0), stop=(ct == 3))
                    ot = hp.tile([128, NSLAB], F16, tag="ot", bufs=4,
                                 name=f"ot_{s}_{jc}")
                    nc.scalar.copy(ot, f_ps)
                    nc.sync.dma_start(
                        out=out_d[jc * 128:(jc + 1) * 128,
                                  s * NSLAB:(s + 1) * NSLAB], in_=ot)

                def stage_qT(s, ct, xs):
                    q_ps = psH.tile([128, NSLAB], F32, tag="fps", bufs=2,
                                    name=f"qps_{s}_{ct}")
                    for dc in range(DC):
                        mm(q_ps, lhsT=wq_sb[:, dc, ct * 128:(ct + 1) * 128],
                           rhs=xs[:, dc, :], start=(dc == 0), stop=(dc == DC - 1))
                    nc.vector.tensor_copy(qt[:, ct, s, :], q_ps)

                # C(s-1) chunk schedule: 8 jc chunks over steps h=3..7 of slab s
                c_sched = {3: [0, 1], 4: [2, 3], 5: [4], 6: [5], 7: [6, 7]}
                # qT(s+2) schedule: 4 ct chunks over steps h=0..3 of slab s
                q_sched = {0: [0], 1: [1], 2: [2], 3: [3]}

                xs_ring = {}
                for t in range(NSTEP):
                    s, h = t // 8, t % 8
                    state[t] = {}
                    if h == 0 and s + 2 < SLABS:
                        xs = hp.tile([128, DC, NSLAB], F32R, tag="xs2",
                                     bufs=2, name=f"xs2_{s + 2}")
                        for dc in range(DC):
                            nc.sync.dma_start(
                                out=xs[:, dc, :],
                                in_=xT_d[dc, :, (s + 2) * NSLAB:(s + 3) * NSLAB])
                        xs_ring[s + 2] = xs
                    state[t]["U_norm"] = stage_nat(t)
                    if t - 2 >= 0:
                        state[t - 2]["UT_sb"] = stage_T(t - 2)
                    if t - 3 >= 0:
                        stage_av(t - 3)
                    if s + 2 < SLABS:
                        for ct in q_sched.get(h, []):
                            stage_qT(s + 2, ct, xs_ring[s + 2])
                    if s >= 1:
                        for jc in c_sched.get(h, []):
                            stage_C(s - 1, jc)
                # epilogue
                for t in (NSTEP - 2, NSTEP - 1):
                    state[t]["UT_sb"] = stage_T(t)
                for t in (NSTEP - 3, NSTEP - 2, NSTEP - 1):
                    stage_av(t)
                for jc in range(8):
                    stage_C(SLABS - 1, jc)

    nc.compile()
    return nc


def kernel(x, W_qkv, E, W_out, b_out):
    x = np.ascontiguousarray(np.asarray(x, dtype=np.float32))
    W_qkv = np.asarray(W_qkv, dtype=np.float32)
    E_np = np.asarray(E, dtype=np.float32)
    W_out = np.asarray(W_out, dtype=np.float32)
    b_out = np.asarray(b_out, dtype=np.float32)

    if "nc" not in _cache:
        _cache["nc"] = build_program()
    nc = _cache["nc"]

    E_t = np.ascontiguousarray(E_np.reshape(NT, 128, KR))
    id16 = np.eye(128, dtype=np.float16)
    in_maps = []
    for core in range(8):
        b, g = core // 2, core % 2
        cols = slice(g * CG, (g + 1) * CG)
        xn_t = x[b].reshape(NT, 128, DIM)
        xT_t = np.ascontiguousarray(x[b].T).reshape(DC, 128, SEQ)
        Wq_t = np.ascontiguousarray(
            (W_qkv[:, 0 * DIM:1 * DIM][:, cols] * SCALE)).reshape(DC, 128, CG)
        Wk_t = np.ascontiguousarray(W_qkv[:, 1 * DIM:2 * DIM][:, cols]).reshape(
            DC, 128, CG)
        Wv_t = np.ascontiguousarray(W_qkv[:, 2 * DIM:3 * DIM][:, cols]).reshape(
            DC, 128, CG)
        Wo_t = np.ascontiguousarray(
            W_out[g * CG:(g + 1) * CG, :].astype(np.float16)).reshape(
            CG // 128, 128, DIM)
        in_maps.append({
            "xn": xn_t, "xT": xT_t, "E": E_t, "Wq": Wq_t, "Wk": Wk_t,
            "Wv": Wv_t, "Wo": Wo_t, "id16": id16,
        })

    trace = bool(int(os.environ.get("KERNEL_TRACE", "0")))
    res = run_bass_kernel_spmd(nc, in_maps, core_ids=list(range(8)), trace=trace)
    _cache["last_results"] = res

    # partials come back transposed [DIM, SEQ] fp16; sum per batch in f32
    accT = np.zeros((4, DIM, SEQ), dtype=np.float32)
    for core in range(8):
        accT[core // 2] += res.results[core]["out"].astype(np.float32)
    out = np.ascontiguousarray(accT.transpose(0, 2, 1))
    out += b_out[None, None, :]
    return out


# revision 17
# speedup vs baseline: 2.7723x; 1.0034x over previous
"""Linformer self-attention on 8 Trainium2 NeuronCores.

Sharding: core = (batch b, head-group g) with b = core//2, g = core%2.
Each core computes attention for batch b and its 8 heads (512 of the 1024
channels), then a row-sharded W_out matmul producing a partial output in
transposed [1024, 4096] fp16 layout; the host sums the two partials per
batch in f32, transposes once, and adds b_out.

Key algebraic restructure vs the naive dataflow (Linformer associativity):
  k_proj = E^T (x Wk) = (E^T x) Wk   and likewise for v_proj.
Computing xE^T = x^T E first (shared by k and v) cuts the projection phase
from ~328k to ~82k PE cycles.

Per-core dataflow:
  FRONT: stream xn (natural [n,d] tiles) + E tiles; accumulate
    xET[d,kr] = sum_n x[n,d] E[n,kr] in PSUM; PE-transpose each xn tile to
    xnT for the q path; every 4 tiles emit one slab's qT = Wq^T x^T
    (Wq pre-scaled by 1/sqrt(hd) on host); then kpT = Wk^T xET and
    vp = xET^T-slices @ Wv.
  HEADS (per slab s, head h, software-pipelined): scores in natural
    layout [n,kr] via one PE pass; row max via free-axis reduce (negated);
    exp on ScalarE with per-partition bias; denominator via free-axis
    reduce_sum; tiny [128,4] reciprocal; normalize U (per-partition
    scalar mult); PE-transpose of normalized U (fp16); AV matmul;
    W_out chunks of the previous slab interleaved between heads.

Precision: q/k score chain fully f32r; U/vp/attn-out/W_out/output fp16.
"""

import os
import numpy as np

import concourse.bacc as bacc
import concourse.tile as tile
from concourse import mybir
from concourse.bass_utils import run_bass_kernel_spmd

F32 = mybir.dt.float32
F16 = mybir.dt.float16
F32R = mybir.dt.float32r
EXP = mybir.ActivationFunctionType.Exp
AXX = mybir.AxisListType.X

DIM, SEQ, KR, HD = 1024, 4096, 256, 64
CG = 512               # channels per head-group (8 heads x 64)
NSLAB = 512
SLABS = SEQ // NSLAB   # 8
NT = SEQ // 128        # 32 natural n-tiles
DC = DIM // 128        # 8 d-tiles
SCALE = HD ** -0.5

_cache = {}


def build_program():
    nc = bacc.Bacc("TRN2", target_bir_lowering=False, debug=False, num_devices=8)

    xn_d = nc.dram_tensor("xn", [NT, 128, DIM], F32R, kind="ExternalInput")
    xT_d = nc.dram_tensor("xT", [DC, 128, SEQ], F32R, kind="ExternalInput")
    Ed = nc.dram_tensor("E", [NT, 128, KR], F32R, kind="ExternalInput")
    Wq = nc.dram_tensor("Wq", [DC, 128, CG], F32R, kind="ExternalInput")
    Wk = nc.dram_tensor("Wk", [DC, 128, CG], F32R, kind="ExternalInput")
    Wv = nc.dram_tensor("Wv", [DC, 128, CG], F32R, kind="ExternalInput")
    Wo = nc.dram_tensor("Wo", [CG // 128, 128, DIM], F16, kind="ExternalInput")
    id32_d = nc.dram_tensor("id32", [128, 128], F32R, kind="ExternalInput")
    id16_d = nc.dram_tensor("id16", [128, 128], F16, kind="ExternalInput")
    out_d = nc.dram_tensor("out", [DIM, SEQ], F16, kind="ExternalOutput")
    dbg = os.environ.get("KERNEL_DEBUG", "0") == "1"
    if dbg:
        dbg_xET = nc.dram_tensor("dbg_xET", [128, DC, KR], F32R, kind="ExternalOutput")
        dbg_kpT = nc.dram_tensor("dbg_kpT", [128, 4, KR], F32R, kind="ExternalOutput")
        dbg_vp = nc.dram_tensor("dbg_vp", [128, 2, CG], F16, kind="ExternalOutput")
        dbg_qt = nc.dram_tensor("dbg_qt", [128, 4, NSLAB], F32R, kind="ExternalOutput")

    mm = nc.tensor.matmul

    with tile.TileContext(nc) as tc:
        with tc.tile_pool(name="const", bufs=1) as const:
            wq_sb = const.tile([128, DC, CG], F32R)
            wo_sb = const.tile([128, 4, DIM], F16)
            id32_sb = const.tile([128, 128], F32R)
            id16_sb = const.tile([128, 128], F16)
            xET_sb = const.tile([128, DC, KR], F32R)   # x^T E  [d, kr]
            kpT_sb = const.tile([128, 4, KR], F32R)    # (kp)^T [c, kr]
            vp_sb = const.tile([128, 2, CG], F16)      # vp     [kr, c]
            qt = const.tile([128, 4, SLABS, NSLAB], F32R)  # q^T, all slabs

            nc.sync.dma_start(out=id32_sb, in_=id32_d[:, :])
            nc.sync.dma_start(out=id16_sb, in_=id16_d[:, :])

            # ---------------- FRONT: xET accumulate + xnT transposes + qT ----
            with tc.tile_pool(name="frA", bufs=1) as frA, \
                 tc.tile_pool(name="psQ", bufs=1, space="PSUM") as psQ:
                wk_sb = frA.tile([128, DC, CG], F32R)
                wv_sb = frA.tile([128, DC, CG], F32R)

                def rr_copy(i, out, in_):
                    eng = (nc.vector.tensor_copy, nc.scalar.copy)[i % 2]
                    eng(out, in_)

                with tc.tile_pool(name="psE", bufs=1, space="PSUM") as psE:
                    # xE natural [kr, d]: 2 krt x 2 dh chunks of [128, 512]
                    xE_ps = psE.tile([128, 2, 2, NSLAB], F32)  # 4 banks
                    for nt in range(NT):
                        xt = frA.tile([128, DIM], F32R, tag="xn", bufs=3,
                                      name=f"xt_{nt}")
                        nc.sync.dma_start(out=xt, in_=xn_d[nt])
                        et = frA.tile([128, KR], F32R, tag="et", bufs=3,
                                      name=f"et_{nt}")
                        nc.sync.dma_start(out=et, in_=Ed[nt])
                        if nt == 0:
                            for dc in range(2):
                                nc.sync.dma_start(out=wq_sb[:, dc, :], in_=Wq[dc])
                        elif nt == 1:
                            for dc in range(2, DC):
                                nc.sync.dma_start(out=wq_sb[:, dc, :], in_=Wq[dc])
                        elif nt == 2:
                            for ct in range(4):
                                nc.sync.dma_start(out=wo_sb[:, ct, :], in_=Wo[ct])
                        elif nt == 3:
                            for dc in range(DC):
                                nc.sync.dma_start(out=wk_sb[:, dc, :], in_=Wk[dc])
                        elif nt == 4:
                            for dc in range(DC):
                                nc.sync.dma_start(out=wv_sb[:, dc, :], in_=Wv[dc])
                        for krt in range(2):
                            for dh in range(2):
                                mm(xE_ps[:, krt, dh, :],
                                   lhsT=et[:, krt * 128:(krt + 1) * 128],
                                   rhs=xt[:, dh * NSLAB:(dh + 1) * NSLAB],
                                   start=(nt == 0), stop=(nt == NT - 1))
                    xE_sb = frA.tile([128, 2, DIM], F32R)
                    for krt in range(2):
                        rr_copy(krt, xE_sb[:, krt, :], xE_ps[:, krt, :, :])
                    # transpose xE [kr, d] -> xET [d, kr]: 16 blocks via PE
                    for grp in range(4):
                        tp = psQ.tile([128, 4, 128], F32R, tag="tp", bufs=2,
                                      name=f"tpx_{grp}")
                        for i in range(4):
                            blk = grp * 4 + i
                            dsub, krt = blk // 2, blk % 2
                            mm(tp[:, i, :],
                               lhsT=xE_sb[:, krt, dsub * 128:(dsub + 1) * 128],
                               rhs=id32_sb, is_transpose=True,
                               start=(i == 0), stop=(i == 3))
                        for i in range(4):
                            blk = grp * 4 + i
                            dsub, krt = blk // 2, blk % 2
                            rr_copy(blk, xET_sb[:, dsub, krt * 128:(krt + 1) * 128],
                                    tp[:, i, :])

                # qT for the first two slabs (rest streams during heads)
                for s in range(2):
                    xs = frA.tile([128, DC, NSLAB], F32R, tag="xs",
                                  bufs=2, name=f"xs_{s}")
                    for dc in range(DC):
                        nc.sync.dma_start(
                            out=xs[:, dc, :],
                            in_=xT_d[dc, :, s * NSLAB:(s + 1) * NSLAB])
                    for ct in range(4):
                        q_ps = psQ.tile([128, NSLAB], F32, tag="qps",
                                        bufs=2, name=f"qps_{s}_{ct}")
                        for dc in range(DC):
                            mm(q_ps,
                               lhsT=wq_sb[:, dc, ct * 128:(ct + 1) * 128],
                               rhs=xs[:, dc, :],
                               start=(dc == 0), stop=(dc == DC - 1))
                        rr_copy(ct, qt[:, ct, s, :], q_ps)

                # kpT / vp from xET (reuses the 4 banks freed by psE)
                with tc.tile_pool(name="psKV", bufs=1, space="PSUM") as psKV:
                    kpT_ps = psKV.tile([128, 4, KR], F32)
                    for dc in range(DC):
                        for ct in range(4):
                            mm(kpT_ps[:, ct, :],
                               lhsT=wk_sb[:, dc, ct * 128:(ct + 1) * 128],
                               rhs=xET_sb[:, dc, :],
                               start=(dc == 0 and ct % 2 == 0),
                               stop=(dc == DC - 1))
                    nc.vector.tensor_copy(kpT_sb, kpT_ps)
                    vp_ps = psKV.tile([128, 2, CG], F32)
                    for dc in range(DC):
                        for krt in range(2):
                            mm(vp_ps[:, krt, :],
                               lhsT=xET_sb[:, dc, krt * 128:(krt + 1) * 128],
                               rhs=wv_sb[:, dc, :],
                               start=(dc == 0), stop=(dc == DC - 1))
                    nc.vector.tensor_copy(vp_sb, vp_ps)

            if dbg:
                nc.sync.dma_start(out=dbg_xET[:, :, :], in_=xET_sb)
                nc.sync.dma_start(out=dbg_kpT[:, :, :], in_=kpT_sb)
                nc.sync.dma_start(out=dbg_vp[:, :, :], in_=vp_sb)
                nc.sync.dma_start(out=dbg_qt[:, :, :], in_=qt[:, :, 0, :])

            # ---------------- HEADS epoch (software-pipelined) --------------
            with tc.tile_pool(name="hp", bufs=1) as hp, \
                 tc.tile_pool(name="psH", bufs=1, space="PSUM") as psH:
                outU = hp.tile([128, 4, 2, NSLAB], F16)  # 2-slab ring

                NSTEP = SLABS * 8
                state = {}

                def stage_nat(t):
                    s, h = t // 8, t % 8
                    hp_, ct_h = (h % 2) * 64, h // 2
                    kph = kpT_sb[hp_:hp_ + 64, ct_h, :]
                    nat = [None, None]
                    mrows = hp.tile([128, 4], F32, tag="mrows", bufs=3,
                                    name=f"mrows_{t}")
                    U_nat = hp.tile([128, 4, KR], F16, tag="unat", bufs=2,
                                    name=f"unat_{t}")
                    for hf in range(2):
                        natp = psH.tile([128, 2, KR], F32, tag="nat", bufs=2,
                                        name=f"nat_{t}_{hf}")
                        for i in range(2):
                            ns = hf * 2 + i
                            qh = qt[hp_:hp_ + 64, ct_h, s,
                                    ns * 128:(ns + 1) * 128]
                            mm(natp[:, i, :], lhsT=qh, rhs=kph,
                               start=(i == 0), stop=(i == 1))
                        nc.vector.reduce_max(mrows[:, 2 * hf:2 * hf + 2], natp,
                                             axis=AXX, negate=True)
                        for i in range(2):
                            ns = hf * 2 + i
                            nc.scalar.activation(U_nat[:, ns, :], natp[:, i, :],
                                                 EXP, bias=mrows[:, ns:ns + 1],
                                                 scale=1.0)
                        nat[hf] = natp
                    denom = hp.tile([128, 4], F16, tag="denom", bufs=2,
                                    name=f"denom_{t}")
                    srecip = hp.tile([128, 4], F32, tag="srecip", bufs=2,
                                     name=f"srecip_{t}")
                    with nc.allow_low_precision(
                            reason="softmax denom in [1,256]; fp16 ok"):
                        nc.vector.reduce_sum(denom, U_nat, axis=AXX)
                        nc.vector.reciprocal(srecip, denom)
                    U_norm = hp.tile([128, 4, KR], F16, tag="unorm", bufs=3,
                                     name=f"unorm_{t}")
                    for ns in range(4):
                        nc.vector.tensor_scalar_mul(U_norm[:, ns, :],
                                                    U_nat[:, ns, :],
                                                    srecip[:, ns:ns + 1])
                    return U_norm

                def stage_T(t):
                    U_norm = state[t]["U_norm"]
                    UT_ps = psH.tile([128, 2, NSLAB], F16, tag="utps", bufs=2,
                                     name=f"utps_{t}")
                    for ns in range(4):
                        for kb in range(2):
                            mm(UT_ps[:, kb, ns * 128:(ns + 1) * 128],
                               lhsT=U_norm[:, ns, kb * 128:(kb + 1) * 128],
                               rhs=id16_sb, is_transpose=True,
                               start=(ns == 0 and kb == 0),
                               stop=(ns == 3 and kb == 1))
                    UT_sb = hp.tile([128, 2, NSLAB], F16, tag="utsb", bufs=2,
                                    name=f"utsb_{t}")
                    if t % 2 == 0:
                        nc.vector.tensor_copy(UT_sb, UT_ps)
                    else:
                        nc.scalar.copy(UT_sb, UT_ps)
                    return UT_sb

                def stage_av(t):
                    s, h = t // 8, t % 8
                    hp_, ct_h = (h % 2) * 64, h // 2
                    UT_sb = state[t]["UT_sb"]
                    av_ps = psH.tile([128, NSLAB], F32, tag="av", bufs=2,
                                     name=f"av_{t}")
                    for krt in range(2):
                        mm(av_ps[hp_:hp_ + 64, :],
                           lhsT=vp_sb[:, krt, h * 64:(h + 1) * 64],
                           rhs=UT_sb[:, krt, :],
                           start=(krt == 0), stop=(krt == 1))
                    nc.scalar.copy(outU[hp_:hp_ + 64, ct_h, s % 2, :],
                                   av_ps[hp_:hp_ + 64, :])

                def stage_C(s, jc):
                    f_ps = psH.tile([128, NSLAB], F32, tag="fps", bufs=2,
                                    name=f"fps_{s}_{jc}")
                    for ct in range(4):
                        mm(f_ps, lhsT=wo_sb[:, ct, jc * 128:(jc + 1) * 128],
                           rhs=outU[:, ct, s % 2, :],
                           start=(ct == 0), stop=(ct == 3))
                    ot = hp.tile([128, NSLAB], F16, tag="ot", bufs=4,
                                 name=f"ot_{s}_{jc}")
                    nc.scalar.copy(ot, f_ps)
                    nc.sync.dma_start(
                        out=out_d[jc * 128:(jc + 1) * 128,
                                  s * NSLAB:(s + 1) * NSLAB], in_=ot)

                def stage_qT(s, ct, xs):
                    q_ps = psH.tile([128, NSLAB], F32, tag="fps", bufs=2,
                                    name=f"qps_{s}_{ct}")
                    for dc in range(DC):
                        mm(q_ps, lhsT=wq_sb[:, dc, ct * 128:(ct + 1) * 128],
                           rhs=xs[:, dc, :], start=(dc == 0), stop=(dc == DC - 1))
                    nc.vector.tensor_copy(qt[:, ct, s, :], q_ps)

                # C(s-1) chunk schedule: 8 jc chunks over steps h=3..7 of slab s
                c_sched = {3: [0, 1], 4: [2, 3], 5: [4], 6: [5], 7: [6, 7]}
                # qT(s+2) schedule: 4 ct chunks over steps h=0..3 of slab s
                q_sched = {0: [0], 1: [1], 2: [2], 3: [3]}

                xs_ring = {}
                for t in range(NSTEP):
                    s, h = t // 8, t % 8
                    state[t] = {}
                    if h == 0 and s + 2 < SLABS:
                        xs = hp.tile([128, DC, NSLAB], F32R, tag="xs2",
                                     bufs=2, name=f"xs2_{s + 2}")
                        for dc in range(DC):
                            nc.sync.dma_start(
                                out=xs[:, dc, :],
                                in_=xT_d[dc, :, (s + 2) * NSLAB:(s + 3) * NSLAB])
                        xs_ring[s + 2] = xs
                    state[t]["U_norm"] = stage_nat(t)
                    if t - 2 >= 0:
                        state[t - 2]["UT_sb"] = stage_T(t - 2)
                    if t - 3 >= 0:
                        stage_av(t - 3)
                    if s + 2 < SLABS:
                        for ct in q_sched.get(h, []):
                            stage_qT(s + 2, ct, xs_ring[s + 2])
                    if s >= 1:
                        for jc in c_sched.get(h, []):
                            stage_C(s - 1, jc)
                # epilogue
                for t in (NSTEP - 2, NSTEP - 1):
                    state[t]["UT_sb"] = stage_T(t)
                for t in (NSTEP - 3, NSTEP - 2, NSTEP - 1):
                    stage_av(t)
                for jc in range(8):
                    stage_C(SLABS - 1, jc)

    nc.compile()
    return nc


def kernel(x, W_qkv, E, W_out, b_out):
    x = np.ascontiguousarray(np.asarray(x, dtype=np.float32))
    W_qkv = np.asarray(W_qkv, dtype=np.float32)
    E_np = np.asarray(E, dtype=np.float32)
    W_out = np.asarray(W_out, dtype=np.float32)
    b_out = np.asarray(b_out, dtype=np.float32)

    if "nc" not in _cache:
        _cache["nc"] = build_program()
    nc = _cache["nc"]

    E_t = np.ascontiguousarray(E_np.reshape(NT, 128, KR))
    id32 = np.eye(128, dtype=np.float32)
    id16 = np.eye(128, dtype=np.float16)
    in_maps = []
    for core in range(8):
        b, g = core // 2, core % 2
        cols = slice(g * CG, (g + 1) * CG)
        xn_t = x[b].reshape(NT, 128, DIM)
        xT_t = np.ascontiguousarray(x[b].T).reshape(DC, 128, SEQ)
        Wq_t = np.ascontiguousarray(
            (W_qkv[:, 0 * DIM:1 * DIM][:, cols] * SCALE)).reshape(DC, 128, CG)
        Wk_t = np.ascontiguousarray(W_qkv[:, 1 * DIM:2 * DIM][:, cols]).reshape(
            DC, 128, CG)
        Wv_t = np.ascontiguousarray(W_qkv[:, 2 * DIM:3 * DIM][:, cols]).reshape(
            DC, 128, CG)
        Wo_t = np.ascontiguousarray(
            W_out[g * CG:(g + 1) * CG, :].astype(np.float16)).reshape(
            CG // 128, 128, DIM)
        in_maps.append({
            "xn": xn_t, "xT": xT_t, "E": E_t, "Wq": Wq_t, "Wk": Wk_t,
            "Wv": Wv_t, "Wo": Wo_t, "id32": id32, "id16": id16,
        })

    trace = bool(int(os.environ.get("KERNEL_TRACE", "0")))
    res = run_bass_kernel_spmd(nc, in_maps, core_ids=list(range(8)), trace=trace)
    _cache["last_results"] = res

    # partials come back transposed [DIM, SEQ] fp16; sum per batch in f32
    accT = np.zeros((4, DIM, SEQ), dtype=np.float32)
    for core in range(8):
        accT[core // 2] += res.results[core]["out"].astype(np.float32)
    out = np.ascontiguousarray(accT.transpose(0, 2, 1))
    out += b_out[None, None, :]
    return out


# revision 18
# speedup vs baseline: 3.1524x; 1.1371x over previous
"""Linformer self-attention on 8 Trainium2 NeuronCores.

Sharding: core = (batch b, head-group g) with b = core//2, g = core%2.
Each core computes attention for batch b and its 8 heads (512 of the 1024
channels), then a row-sharded W_out matmul producing a partial output in
transposed [1024, 4096] fp16 layout; the host sums the two partials per
batch in f32, transposes once, and adds b_out.

Key algebraic restructure vs the naive dataflow (Linformer associativity):
  k_proj = E^T (x Wk) = (E^T x) Wk   and likewise for v_proj.
Computing xE^T = x^T E first (shared by k and v) cuts the projection phase
from ~328k to ~82k PE cycles.

Per-core dataflow:
  FRONT: stream xn (natural [n,d] tiles) + E tiles; accumulate
    xET[d,kr] = sum_n x[n,d] E[n,kr] in PSUM; PE-transpose each xn tile to
    xnT for the q path; every 4 tiles emit one slab's qT = Wq^T x^T
    (Wq pre-scaled by 1/sqrt(hd) on host); then kpT = Wk^T xET and
    vp = xET^T-slices @ Wv.
  HEADS (per slab s, head h, software-pipelined): scores in natural
    layout [n,kr] via one PE pass; row max via free-axis reduce (negated);
    exp on ScalarE with per-partition bias; denominator via free-axis
    reduce_sum; tiny [128,4] reciprocal; normalize U (per-partition
    scalar mult); PE-transpose of normalized U (fp16); AV matmul;
    W_out chunks of the previous slab interleaved between heads.

Precision: q/k score chain fully f32r; U/vp/attn-out/W_out/output fp16.
"""

import os
import numpy as np

import concourse.bacc as bacc
import concourse.tile as tile
from concourse import mybir
from concourse.bass_utils import run_bass_kernel_spmd

F32 = mybir.dt.float32
F16 = mybir.dt.float16
F32R = mybir.dt.float32r
EXP = mybir.ActivationFunctionType.Exp
AXX = mybir.AxisListType.X

DIM, SEQ, KR, HD = 1024, 4096, 256, 64
CG = 512               # channels per head-group (8 heads x 64)
NSLAB = 512
SLABS = SEQ // NSLAB   # 8
NT = SEQ // 128        # 32 natural n-tiles
DC = DIM // 128        # 8 d-tiles
SCALE = HD ** -0.5

_cache = {}


def build_program():
    nc = bacc.Bacc("TRN2", target_bir_lowering=False, debug=False, num_devices=8)

    xn_d = nc.dram_tensor("xn", [NT, 128, DIM], F32R, kind="ExternalInput")
    xT_d = nc.dram_tensor("xT", [DC, 128, SEQ], F16, kind="ExternalInput")
    Ed = nc.dram_tensor("E", [NT, 128, KR], F32R, kind="ExternalInput")
    Wq = nc.dram_tensor("Wq", [DC, 128, CG], F16, kind="ExternalInput")
    Wk = nc.dram_tensor("Wk", [DC, 128, CG], F32R, kind="ExternalInput")
    Wv = nc.dram_tensor("Wv", [DC, 128, CG], F32R, kind="ExternalInput")
    Wo = nc.dram_tensor("Wo", [CG // 128, 128, DIM], F16, kind="ExternalInput")
    id32_d = nc.dram_tensor("id32", [128, 128], F32R, kind="ExternalInput")
    id16_d = nc.dram_tensor("id16", [128, 128], F16, kind="ExternalInput")
    out_d = nc.dram_tensor("out", [DIM, SEQ], F16, kind="ExternalOutput")
    dbg = os.environ.get("KERNEL_DEBUG", "0") == "1"
    if dbg:
        dbg_xET = nc.dram_tensor("dbg_xET", [128, DC, KR], F32R, kind="ExternalOutput")
        dbg_kpT = nc.dram_tensor("dbg_kpT", [128, 4, KR], F32R, kind="ExternalOutput")
        dbg_vp = nc.dram_tensor("dbg_vp", [128, 2, CG], F16, kind="ExternalOutput")
        dbg_qt = nc.dram_tensor("dbg_qt", [128, 4, NSLAB], F32R, kind="ExternalOutput")

    mm = nc.tensor.matmul

    with tile.TileContext(nc) as tc:
        with tc.tile_pool(name="const", bufs=1) as const:
            wq_sb = const.tile([128, DC, CG], F16)
            wo_sb = const.tile([128, 4, DIM], F16)
            id32_sb = const.tile([128, 128], F32R)
            id16_sb = const.tile([128, 128], F16)
            xET_sb = const.tile([128, DC, KR], F32R)   # x^T E  [d, kr]
            kpT_sb = const.tile([128, 4, KR], F32R)    # (kp)^T [c, kr]
            vp_sb = const.tile([128, 2, CG], F16)      # vp     [kr, c]
            qt = const.tile([128, 4, SLABS, NSLAB], F32R)  # q^T, all slabs

            nc.sync.dma_start(out=id32_sb, in_=id32_d[:, :])
            nc.sync.dma_start(out=id16_sb, in_=id16_d[:, :])

            # ---------------- FRONT: xET accumulate + xnT transposes + qT ----
            with tc.tile_pool(name="frA", bufs=1) as frA, \
                 tc.tile_pool(name="psQ", bufs=1, space="PSUM") as psQ:
                wk_sb = frA.tile([128, DC, CG], F32R)
                wv_sb = frA.tile([128, DC, CG], F32R)

                def rr_copy(i, out, in_):
                    eng = (nc.vector.tensor_copy, nc.scalar.copy)[i % 2]
                    eng(out, in_)

                with tc.tile_pool(name="psE", bufs=1, space="PSUM") as psE:
                    # xE natural [kr, d]: 2 krt x 2 dh chunks of [128, 512]
                    xE_ps = psE.tile([128, 2, 2, NSLAB], F32)  # 4 banks
                    for nt in range(NT):
                        xt = frA.tile([128, DIM], F32R, tag="xn", bufs=3,
                                      name=f"xt_{nt}")
                        nc.sync.dma_start(out=xt, in_=xn_d[nt])
                        et = frA.tile([128, KR], F32R, tag="et", bufs=3,
                                      name=f"et_{nt}")
                        nc.sync.dma_start(out=et, in_=Ed[nt])
                        if nt == 0:
                            for dc in range(2):
                                nc.sync.dma_start(out=wq_sb[:, dc, :], in_=Wq[dc])
                        elif nt == 1:
                            for dc in range(2, DC):
                                nc.sync.dma_start(out=wq_sb[:, dc, :], in_=Wq[dc])
                        elif nt == 2:
                            for ct in range(4):
                                nc.sync.dma_start(out=wo_sb[:, ct, :], in_=Wo[ct])
                        elif nt == 3:
                            for dc in range(DC):
                                nc.sync.dma_start(out=wk_sb[:, dc, :], in_=Wk[dc])
                        elif nt == 4:
                            for dc in range(DC):
                                nc.sync.dma_start(out=wv_sb[:, dc, :], in_=Wv[dc])
                        for krt in range(2):
                            for dh in range(2):
                                mm(xE_ps[:, krt, dh, :],
                                   lhsT=et[:, krt * 128:(krt + 1) * 128],
                                   rhs=xt[:, dh * NSLAB:(dh + 1) * NSLAB],
                                   start=(nt == 0), stop=(nt == NT - 1))
                    xE_sb = frA.tile([128, 2, DIM], F32R)
                    for krt in range(2):
                        rr_copy(krt, xE_sb[:, krt, :], xE_ps[:, krt, :, :])
                    # transpose xE [kr, d] -> xET [d, kr]: 16 blocks via PE
                    for grp in range(4):
                        tp = psQ.tile([128, 4, 128], F32R, tag="tp", bufs=2,
                                      name=f"tpx_{grp}")
                        for i in range(4):
                            blk = grp * 4 + i
                            dsub, krt = blk // 2, blk % 2
                            mm(tp[:, i, :],
                               lhsT=xE_sb[:, krt, dsub * 128:(dsub + 1) * 128],
                               rhs=id32_sb, is_transpose=True,
                               start=(i == 0), stop=(i == 3))
                        for i in range(4):
                            blk = grp * 4 + i
                            dsub, krt = blk // 2, blk % 2
                            rr_copy(blk, xET_sb[:, dsub, krt * 128:(krt + 1) * 128],
                                    tp[:, i, :])

                # qT for the first two slabs (rest streams during heads)
                for s in range(2):
                    xs = frA.tile([128, DC, NSLAB], F16, tag="xs",
                                  bufs=2, name=f"xs_{s}")
                    for dc in range(DC):
                        nc.sync.dma_start(
                            out=xs[:, dc, :],
                            in_=xT_d[dc, :, s * NSLAB:(s + 1) * NSLAB])
                    for ct in range(4):
                        q_ps = psQ.tile([128, NSLAB], F32, tag="qps",
                                        bufs=2, name=f"qps_{s}_{ct}")
                        for dc in range(DC):
                            mm(q_ps,
                               lhsT=wq_sb[:, dc, ct * 128:(ct + 1) * 128],
                               rhs=xs[:, dc, :],
                               start=(dc == 0), stop=(dc == DC - 1))
                        rr_copy(ct, qt[:, ct, s, :], q_ps)

                # kpT / vp from xET (reuses the 4 banks freed by psE)
                with tc.tile_pool(name="psKV", bufs=1, space="PSUM") as psKV:
                    kpT_ps = psKV.tile([128, 4, KR], F32)
                    for dc in range(DC):
                        for ct in range(4):
                            mm(kpT_ps[:, ct, :],
                               lhsT=wk_sb[:, dc, ct * 128:(ct + 1) * 128],
                               rhs=xET_sb[:, dc, :],
                               start=(dc == 0 and ct % 2 == 0),
                               stop=(dc == DC - 1))
                    nc.vector.tensor_copy(kpT_sb, kpT_ps)
                    vp_ps = psKV.tile([128, 2, CG], F32)
                    for dc in range(DC):
                        for krt in range(2):
                            mm(vp_ps[:, krt, :],
                               lhsT=xET_sb[:, dc, krt * 128:(krt + 1) * 128],
                               rhs=wv_sb[:, dc, :],
                               start=(dc == 0), stop=(dc == DC - 1))
                    nc.vector.tensor_copy(vp_sb, vp_ps)

            if dbg:
                nc.sync.dma_start(out=dbg_xET[:, :, :], in_=xET_sb)
                nc.sync.dma_start(out=dbg_kpT[:, :, :], in_=kpT_sb)
                nc.sync.dma_start(out=dbg_vp[:, :, :], in_=vp_sb)
                nc.sync.dma_start(out=dbg_qt[:, :, :], in_=qt[:, :, 0, :])

            # ---------------- HEADS epoch (software-pipelined) --------------
            with tc.tile_pool(name="hp", bufs=1) as hp, \
                 tc.tile_pool(name="psH", bufs=1, space="PSUM") as psH:
                outU = hp.tile([128, 4, 2, NSLAB], F16)  # 2-slab ring

                NSTEP = SLABS * 8
                state = {}

                def stage_nat(t):
                    s, h = t // 8, t % 8
                    hp_, ct_h = (h % 2) * 64, h // 2
                    kph = kpT_sb[hp_:hp_ + 64, ct_h, :]
                    nat = [None, None]
                    mrows = hp.tile([128, 4], F32, tag="mrows", bufs=3,
                                    name=f"mrows_{t}")
                    U_nat = hp.tile([128, 4, KR], F16, tag="unat", bufs=2,
                                    name=f"unat_{t}")
                    for hf in range(2):
                        natp = psH.tile([128, 2, KR], F32, tag="nat", bufs=2,
                                        name=f"nat_{t}_{hf}")
                        for i in range(2):
                            ns = hf * 2 + i
                            qh = qt[hp_:hp_ + 64, ct_h, s,
                                    ns * 128:(ns + 1) * 128]
                            mm(natp[:, i, :], lhsT=qh, rhs=kph,
                               start=(i == 0), stop=(i == 1))
                        nc.vector.reduce_max(mrows[:, 2 * hf:2 * hf + 2], natp,
                                             axis=AXX, negate=True)
                        for i in range(2):
                            ns = hf * 2 + i
                            nc.scalar.activation(U_nat[:, ns, :], natp[:, i, :],
                                                 EXP, bias=mrows[:, ns:ns + 1],
                                                 scale=1.0)
                        nat[hf] = natp
                    denom = hp.tile([128, 4], F16, tag="denom", bufs=2,
                                    name=f"denom_{t}")
                    srecip = hp.tile([128, 4], F32, tag="srecip", bufs=2,
                                     name=f"srecip_{t}")
                    with nc.allow_low_precision(
                            reason="softmax denom in [1,256]; fp16 ok"):
                        nc.vector.reduce_sum(denom, U_nat, axis=AXX)
                        nc.vector.reciprocal(srecip, denom)
                    U_norm = hp.tile([128, 4, KR], F16, tag="unorm", bufs=3,
                                     name=f"unorm_{t}")
                    for ns in range(4):
                        nc.vector.tensor_scalar_mul(U_norm[:, ns, :],
                                                    U_nat[:, ns, :],
                                                    srecip[:, ns:ns + 1])
                    return U_norm

                def stage_T(t):
                    U_norm = state[t]["U_norm"]
                    UT_ps = psH.tile([128, 2, NSLAB], F16, tag="utps", bufs=2,
                                     name=f"utps_{t}")
                    for ns in range(4):
                        for kb in range(2):
                            mm(UT_ps[:, kb, ns * 128:(ns + 1) * 128],
                               lhsT=U_norm[:, ns, kb * 128:(kb + 1) * 128],
                               rhs=id16_sb, is_transpose=True,
                               start=(ns == 0 and kb == 0),
                               stop=(ns == 3 and kb == 1))
                    UT_sb = hp.tile([128, 2, NSLAB], F16, tag="utsb", bufs=2,
                                    name=f"utsb_{t}")
                    nc.scalar.copy(UT_sb, UT_ps)
                    return UT_sb

                def stage_av(t):
                    s, h = t // 8, t % 8
                    hp_, ct_h = (h % 2) * 64, h // 2
                    UT_sb = state[t]["UT_sb"]
                    av_ps = psH.tile([128, NSLAB], F32, tag="av", bufs=2,
                                     name=f"av_{t}")
                    for krt in range(2):
                        mm(av_ps[hp_:hp_ + 64, :],
                           lhsT=vp_sb[:, krt, h * 64:(h + 1) * 64],
                           rhs=UT_sb[:, krt, :],
                           start=(krt == 0), stop=(krt == 1))
                    nc.scalar.copy(outU[hp_:hp_ + 64, ct_h, s % 2, :],
                                   av_ps[hp_:hp_ + 64, :])

                def stage_C(s, jc):
                    f_ps = psH.tile([128, NSLAB], F32, tag="fps", bufs=2,
                                    name=f"fps_{s}_{jc}")
                    for ct in range(4):
                        mm(f_ps, lhsT=wo_sb[:, ct, jc * 128:(jc + 1) * 128],
                           rhs=outU[:, ct, s % 2, :],
                           start=(ct == 0), stop=(ct == 3))
                    ot = hp.tile([128, NSLAB], F16, tag="ot", bufs=4,
                                 name=f"ot_{s}_{jc}")
                    nc.scalar.copy(ot, f_ps)
                    nc.sync.dma_start(
                        out=out_d[jc * 128:(jc + 1) * 128,
                                  s * NSLAB:(s + 1) * NSLAB], in_=ot)

                def stage_qT(s, ct, xs):
                    q_ps = psH.tile([128, NSLAB], F32, tag="fps", bufs=2,
                                    name=f"qps_{s}_{ct}")
                    for dc in range(DC):
                        mm(q_ps, lhsT=wq_sb[:, dc, ct * 128:(ct + 1) * 128],
                           rhs=xs[:, dc, :], start=(dc == 0), stop=(dc == DC - 1))
                    nc.vector.tensor_copy(qt[:, ct, s, :], q_ps)

                # C(s-1) chunk schedule: 8 jc chunks over steps h=3..7 of slab s
                c_sched = {3: [0, 1], 4: [2, 3], 5: [4], 6: [5], 7: [6, 7]}
                # qT(s+2) schedule: 4 ct chunks over steps h=0..3 of slab s
                q_sched = {0: [0], 1: [1], 2: [2], 3: [3]}

                xs_ring = {}
                for t in range(NSTEP):
                    s, h = t // 8, t % 8
                    state[t] = {}
                    if h == 0 and s + 2 < SLABS:
                        xs = hp.tile([128, DC, NSLAB], F16, tag="xs2",
                                     bufs=2, name=f"xs2_{s + 2}")
                        for dc in range(DC):
                            nc.sync.dma_start(
                                out=xs[:, dc, :],
                                in_=xT_d[dc, :, (s + 2) * NSLAB:(s + 3) * NSLAB])
                        xs_ring[s + 2] = xs
                    state[t]["U_norm"] = stage_nat(t)
                    if t - 2 >= 0:
                        state[t - 2]["UT_sb"] = stage_T(t - 2)
                    if t - 3 >= 0:
                        stage_av(t - 3)
                    if s + 2 < SLABS:
                        for ct in q_sched.get(h, []):
                            stage_qT(s + 2, ct, xs_ring[s + 2])
                    if s >= 1:
                        for jc in c_sched.get(h, []):
                            stage_C(s - 1, jc)
                # epilogue
                for t in (NSTEP - 2, NSTEP - 1):
                    state[t]["UT_sb"] = stage_T(t)
                for t in (NSTEP - 3, NSTEP - 2, NSTEP - 1):
                    stage_av(t)
                for jc in range(8):
                    stage_C(SLABS - 1, jc)

    nc.compile()
    return nc


def kernel(x, W_qkv, E, W_out, b_out):
    x = np.ascontiguousarray(np.asarray(x, dtype=np.float32))
    W_qkv = np.asarray(W_qkv, dtype=np.float32)
    E_np = np.asarray(E, dtype=np.float32)
    W_out = np.asarray(W_out, dtype=np.float32)
    b_out = np.asarray(b_out, dtype=np.float32)

    if "nc" not in _cache:
        _cache["nc"] = build_program()
    nc = _cache["nc"]

    E_t = np.ascontiguousarray(E_np.reshape(NT, 128, KR))
    id32 = np.eye(128, dtype=np.float32)
    id16 = np.eye(128, dtype=np.float16)
    in_maps = []
    for core in range(8):
        b, g = core // 2, core % 2
        cols = slice(g * CG, (g + 1) * CG)
        xn_t = x[b].reshape(NT, 128, DIM)
        xT_t = np.ascontiguousarray(x[b].T.astype(np.float16)).reshape(
            DC, 128, SEQ)
        Wq_t = np.ascontiguousarray(
            (W_qkv[:, 0 * DIM:1 * DIM][:, cols] * SCALE).astype(
                np.float16)).reshape(DC, 128, CG)
        Wk_t = np.ascontiguousarray(W_qkv[:, 1 * DIM:2 * DIM][:, cols]).reshape(
            DC, 128, CG)
        Wv_t = np.ascontiguousarray(W_qkv[:, 2 * DIM:3 * DIM][:, cols]).reshape(
            DC, 128, CG)
        Wo_t = np.ascontiguousarray(
            W_out[g * CG:(g + 1) * CG, :].astype(np.float16)).reshape(
            CG // 128, 128, DIM)
        in_maps.append({
            "xn": xn_t, "xT": xT_t, "E": E_t, "Wq": Wq_t, "Wk": Wk_t,
            "Wv": Wv_t, "Wo": Wo_t, "id32": id32, "id16": id16,
        })

    trace = bool(int(os.environ.get("KERNEL_TRACE", "0")))
    res = run_bass_kernel_spmd(nc, in_maps, core_ids=list(range(8)), trace=trace)
    _cache["last_results"] = res

    # partials come back transposed [DIM, SEQ] fp16; sum per batch in f32
    accT = np.zeros((4, DIM, SEQ), dtype=np.float32)
    for core in range(8):
        accT[core // 2] += res.results[core]["out"].astype(np.float32)
    out = np.ascontiguousarray(accT.transpose(0, 2, 1))
    out += b_out[None, None, :]
    return out
